# revision 11
# baseline (speedup 1.0000x reference)
"""NNConv+GRU message-passing network (ConvGRU) on 8 Trainium2 NeuronCores.

Strategy (v2, tuned from trace analysis of the v1 baseline):
  - Edges sharded by OWNER OF DST node (8 node ranges of 1024); scatter-add
    realized as matmul against a 0/1 selection matrix (exact dup handling).
  - h node-sharded for the GRU; AllGathered (fp16) once per conv layer.
    Edges whose SRC is also core-local are ordered first and gather h from
    the local copy, giving DVE work during the AllGather latency.
  - Per-edge weights We: PE computes hid@w2p into PSUM fp32 (fp16 inputs),
    ACT evacuates to one fp16 [128,4096] SBUF tile per edge-tile, then the
    per-edge matvec is: one broadcast multiply (DVE 2x mode, or GPSIMD for
    a subset of tiles to balance engines), three in-place strided fold-adds
    (DVE 2x), and one short tensor_reduce -> fp16 msg.
  - Everything on the h path is fp16 (fp32 matmuls cost 4 cyc/col vs 1).
  - GRU r/z: the wih@x and whh@h matmuls accumulate into one PSUM tile.

Self-contained: only needs numpy + the concourse/bass stack installed in the
container. All shapes hardcoded for this problem size.
"""
import numpy as np

DIM = 64
DEPTHS = 3
N_NODES = 8192
N_EDGES = 16384
N_GRAPHS = 64
NC = 8
NPC = N_NODES // NC   # 1024 nodes per core
P = 128

TRACE = False
LAST_EXEC_NS = None
LAST_RESULTS = None

_CACHE = {}

# tiles t (past the own-src block) with t % 8 in this set do their broadcast
# multiply on GPSIMD instead of DVE (engine balancing)
GPS_PAT = (1, 3, 5, 7)


def _build(T, T_OWN, b2_zero):
    """Build the (shared) 8-core SPMD program. Per-core data arrives via inputs."""
    import concourse.mybir as mybir
    import concourse.tile as tile
    from concourse import bacc
    import concourse.bass as bass
    from concourse.masks import make_identity

    f32 = mybir.dt.float32
    f16 = mybir.dt.float16
    i32 = mybir.dt.int32
    AF = mybir.ActivationFunctionType
    OP = mybir.AluOpType
    EP = T * P  # padded edge count per core

    nc = bacc.Bacc("TRN2", target_bir_lowering=False, debug=False, num_devices=NC)

    def din(name, shape, dt=f32):
        return nc.dram_tensor(name, shape, dt, kind="ExternalInput")

    xT_d = din("xT", [40, NPC], f16)
    eaT_d = din("eaT", [10, EP], f16)
    srcx_d = din("srcidx", [P, T], i32)
    S_d = din("S", [P, T * NPC], f16)
    pS_d = din("poolS", [NPC, N_GRAPHS], f16)
    fc0_wT_d = din("fc0_wT", [40, 32], f16)
    fc0_b_d = din("fc0_b", [32, 1])
    g0_wihT_d = din("g0_wihT", [32, 192], f16)
    g0_brz_d = din("g0_brz", [128, 1])
    g0_bihn_d = din("g0_bihn", [64, 1])
    g0_bhhn_d = din("g0_bhhn", [64, 1])
    w2p_d = [din(f"w2p{d}", [128, 4096], f16) for d in range(DEPTHS)]
    m1wT_d = [din(f"m1wT{d}", [10, 128], f16) for d in range(DEPTHS)]
    m1b_d = [din(f"m1b{d}", [128, 1]) for d in range(DEPTHS)]
    root_d = [din(f"root{d}", [64, 64], f16) for d in range(DEPTHS)]
    convb_d = [din(f"convb{d}", [64, 1]) for d in range(DEPTHS)]
    wihT_d = [din(f"wihT{d}", [64, 192], f16) for d in range(DEPTHS)]
    whhT_d = [din(f"whhT{d}", [64, 192], f16) for d in range(DEPTHS)]
    brz_d = [din(f"brz{d}", [128, 1]) for d in range(DEPTHS)]
    bihn_d = [din(f"bihn{d}", [64, 1]) for d in range(DEPTHS)]
    bhhn_d = [din(f"bhhn{d}", [64, 1]) for d in range(DEPTHS)]
    b2bc_d = None if b2_zero else [din(f"b2bc{d}", [128, 4096], f16) for d in range(DEPTHS)]
    o0wT_d = din("o0wT", [64, 64], f16)
    o0b_d = din("o0b", [64, 1])
    o1wT_d = din("o1wT", [64, 32], f16)
    o1b_d = din("o1b", [32, 1])
    o2wT_d = din("o2wT", [32, 1], f16)
    o2b_d = din("o2b", [1, 1])

    y_d = nc.dram_tensor("y", [1, N_GRAPHS], f32, kind="ExternalOutput")

    RG = [list(range(NC))]

    with nc.allow_low_precision("fp16 pipeline; final tolerance is 2e-2"), \
         tile.TileContext(nc) as tc:
        with (
            tc.tile_pool(name="const", bufs=1) as cp,
            tc.tile_pool(name="work", bufs=2) as wp,
            tc.tile_pool(name="wsbp", bufs=6) as wsbp,
            tc.tile_pool(name="edge", bufs=2) as ep,
            tc.tile_pool(name="hsfp", bufs=T + 3) as hsfp,
            tc.tile_pool(name="pwe", bufs=2, space="PSUM") as pwe,
            tc.tile_pool(name="pagg", bufs=1, space="PSUM") as pagg,
            tc.tile_pool(name="ptp", bufs=2, space="PSUM") as ptp,
            tc.tile_pool(name="dram", bufs=1, space="DRAM") as dp,
        ):
            # ---------------- constants to SBUF ----------------
            def load(name, dram, shape, dt=f32, ap=None):
                t = cp.tile(shape, dt, name=name)
                nc.sync.dma_start(t[:], dram[:, :] if ap is None else ap)
                return t

            # load order = DMA issue order: phase0 + edge-MLP inputs first so
            # compute starts while the big S / w2p / GRU tensors stream in.
            xT = load("xT_s", xT_d, [40, NPC], f16)
            fc0_wT = load("fc0_wT_s", fc0_wT_d, [40, 32], f16)
            fc0_b = load("fc0_b_s", fc0_b_d, [32, 1])
            g0_wihT = load("g0_wihT_s", g0_wihT_d, [32, 192], f16)
            g0_brz = load("g0_brz_s", g0_brz_d, [128, 1])
            g0_bihn = load("g0_bihn_s", g0_bihn_d, [64, 1])
            g0_bhhn = load("g0_bhhn_s", g0_bhhn_d, [64, 1])
            eaT = load("eaT_s", eaT_d, [10, EP], f16)
            m1wT = [load(f"m1wT_s{d}", m1wT_d[d], [10, 128], f16) for d in range(DEPTHS)]
            m1b = [load(f"m1b_s{d}", m1b_d[d], [128, 1]) for d in range(DEPTHS)]
            srcx = load("srcx_s", srcx_d, [P, T], i32)
            w2p = [load(f"w2p_s{d}", w2p_d[d], [128, 4096], f16) for d in range(DEPTHS)]
            S = cp.tile([P, T * NPC], f16, name="S_s")
            for t in range(T):
                nc.sync.dma_start(
                    S[:, t * NPC:(t + 1) * NPC], S_d[:, t * NPC:(t + 1) * NPC]
                )
            pS = cp.tile([P, 8 * N_GRAPHS], f16, name="pS_s")
            for c in range(8):
                nc.sync.dma_start(
                    pS[:, c * N_GRAPHS:(c + 1) * N_GRAPHS],
                    pS_d[c * P:(c + 1) * P, :],
                )
            rootw = [load(f"root_s{d}", root_d[d], [64, 64], f16) for d in range(DEPTHS)]
            convb = [load(f"convb_s{d}", convb_d[d], [64, 1]) for d in range(DEPTHS)]
            wihT = [load(f"wihT_s{d}", wihT_d[d], [64, 192], f16) for d in range(DEPTHS)]
            whhT = [load(f"whhT_s{d}", whhT_d[d], [64, 192], f16) for d in range(DEPTHS)]
            brz = [load(f"brz_s{d}", brz_d[d], [128, 1]) for d in range(DEPTHS)]
            bihn = [load(f"bihn_s{d}", bihn_d[d], [64, 1]) for d in range(DEPTHS)]
            bhhn = [load(f"bhhn_s{d}", bhhn_d[d], [64, 1]) for d in range(DEPTHS)]
            b2bc = (
                None if b2_zero else
                [load(f"b2bc_s{d}", b2bc_d[d], [128, 4096], f16) for d in range(DEPTHS)]
            )
            o0wT = load("o0wT_s", o0wT_d, [64, 64], f16)
            o0b = load("o0b_s", o0b_d, [64, 1])
            o1wT = load("o1wT_s", o1wT_d, [64, 32], f16)
            o1b = load("o1b_s", o1b_d, [32, 1])
            o2wT = load("o2wT_s", o2wT_d, [32, 1], f16)
            o2b = load("o2b_s", o2b_d, [1, 1])

            ident = cp.tile([64, 64], f16, name="ident")
            make_identity(nc, ident[:])

            hown = [dp.tile([NPC, DIM], f16, name=f"hown{d}") for d in range(DEPTHS)]
            hfull = [dp.tile([N_NODES, DIM], f16, name=f"hfull{d}") for d in range(DEPTHS)]
            ar_in = dp.tile([DIM, N_GRAPHS], f32, name="ar_in")
            ar_out = dp.tile([DIM, N_GRAPHS], f32, name="ar_out")

            # ---------------- helpers ----------------
            def mm512(out_ap_fn, lhsT, rhs_fn, n_total, start, stop):
                """matmuls in 512-wide chunks: out[:, s] = lhsT.T @ rhs[:, s]."""
                off = 0
                while off < n_total:
                    n = min(512, n_total - off)
                    nc.tensor.matmul(
                        out_ap_fn(off, n), lhsT, rhs_fn(off, n),
                        start=start, stop=stop,
                    )
                    off += n

            def gru_elem(rz_s, gi_n_s, hn_s, h_prev, tagp):
                """rz_s [128,1024] f16 (r||z post-sigmoid), gi_n_s/hn_s [64,1024] f16.
                Returns new h_T [64,1024] f16: h' = n + z*(h - n)."""
                # DVE needs equal base partitions for SBUF+SBUF tensor_tensor,
                # so shift the z half down to a base-0 tile via SBUF->SBUF DMA.
                z_s = wp.tile([64, NPC], f16, name=f"z_{tagp}", tag="gru_z")
                nc.sync.dma_start(z_s[:], rz_s[64:128, :])
                t1 = wp.tile([64, NPC], f16, name=f"t1_{tagp}", tag="gru_t1")
                nc.vector.tensor_tensor(out=t1[:], in0=rz_s[0:64, :], in1=hn_s[:], op=OP.mult)
                nc.vector.tensor_tensor(out=t1[:], in0=t1[:], in1=gi_n_s[:], op=OP.add)
                nt = wp.tile([64, NPC], f16, name=f"nt_{tagp}", tag="gru_nt")
                nc.scalar.activation(nt[:], t1[:], AF.Tanh)
                hm = wp.tile([64, NPC], f16, name=f"hm_{tagp}", tag="gru_hm")
                if h_prev is None:
                    # h=0: h' = n - z*n
                    nc.vector.tensor_tensor(out=hm[:], in0=z_s[:], in1=nt[:], op=OP.mult)
                    hnew = wp.tile([64, NPC], f16, name=f"h_{tagp}", tag="hT")
                    nc.vector.tensor_tensor(out=hnew[:], in0=nt[:], in1=hm[:], op=OP.subtract)
                else:
                    nc.vector.tensor_tensor(out=hm[:], in0=h_prev[:], in1=nt[:], op=OP.subtract)
                    nc.vector.tensor_tensor(out=hm[:], in0=hm[:], in1=z_s[:], op=OP.mult)
                    hnew = wp.tile([64, NPC], f16, name=f"h_{tagp}", tag="hT")
                    nc.vector.tensor_tensor(out=hnew[:], in0=hm[:], in1=nt[:], op=OP.add)
                return hnew

            def h_transposes(h_T, d_out, tagp, want_sbuf):
                """PE-transpose h_T [64,1024] f16 -> 8 [128,64] node-major SBUF
                tiles (PSUM can't feed DMA directly), DMA each to hown[d_out]."""
                sb = []
                for c in range(8):
                    tp = ptp.tile([P, DIM], f16, name=f"tp_{tagp}_{c}", tag="tp")
                    nc.tensor.transpose(
                        out=tp[:], in_=h_T[:, c * P:(c + 1) * P], identity=ident[:]
                    )
                    hm = wp.tile([P, DIM], f16, name=f"hnm_{tagp}_{c}", tag=f"hnm{c}")
                    if c % 2 == 0:
                        nc.scalar.activation(hm[:], tp[:], AF.Copy)
                    else:
                        nc.vector.tensor_copy(hm[:], tp[:])
                    if d_out is not None:
                        nc.sync.dma_start(hown[d_out][c * P:(c + 1) * P, :], hm[:])
                    if want_sbuf:
                        sb.append(hm)
                return sb

            # ---------------- edge-MLP hidden states, all depths upfront ----
            hidT = []
            for d in range(DEPTHS):
                ht = cp.tile([P, EP], f16, name=f"hidT{d}")
                off = 0
                while off < EP:
                    n = min(1024, EP - off)
                    hp = pwe.tile([P, NPC], f32, name=f"hid_ps{d}_{off}", tag="pwe")
                    mm512(lambda o, nn, _b=off: hp[:, o:o + nn], m1wT[d][:],
                          lambda o, nn, _b=off: eaT[:, _b + o:_b + o + nn], n, True, True)
                    nc.scalar.activation(
                        ht[:, off:off + n], hp[:, 0:n], AF.Relu, bias=m1b[d][:, 0:1]
                    )
                    off += n
                hidT.append(ht)

            # ---------------- phase 0: fc0 + gru0 (h0 = 0) ----------------
            x0_ps = pwe.tile([P, NPC], f32, name="x0_ps", tag="pwe")
            mm512(lambda o, n: x0_ps[0:32, o:o + n], fc0_wT[:],
                  lambda o, n: xT[:, o:o + n], NPC, True, True)
            x0r = wp.tile([32, NPC], f16, name="x0r")
            nc.scalar.activation(x0r[:], x0_ps[0:32, :], AF.Relu, bias=fc0_b[:, 0:1])

            g0rz_ps = pwe.tile([P, NPC], f32, name="g0rz_ps", tag="pwe")
            mm512(lambda o, n: g0rz_ps[0:128, o:o + n], g0_wihT[:, 0:128],
                  lambda o, n: x0r[:, o:o + n], NPC, True, True)
            rz0 = wp.tile([P, NPC], f16, name="rz0", tag="gru_rz")
            nc.scalar.activation(rz0[:], g0rz_ps[0:128, :], AF.Sigmoid, bias=g0_brz[:, 0:1])

            g0n_ps = pwe.tile([P, NPC], f32, name="g0n_ps", tag="pwe")
            mm512(lambda o, n: g0n_ps[0:64, o:o + n], g0_wihT[:, 128:192],
                  lambda o, n: x0r[:, o:o + n], NPC, True, True)
            gin0 = wp.tile([64, NPC], f16, name="gin0", tag="gru_gin")
            nc.scalar.activation(gin0[:], g0n_ps[0:64, :], AF.Identity, bias=g0_bihn[:, 0:1])
            # h=0 so gh_n = bhh_n: broadcast bhh_n across columns (scale=0 trick)
            hn0 = wp.tile([64, NPC], f16, name="hn0", tag="gru_hn")
            nc.scalar.activation(hn0[:], gin0[:], AF.Identity, bias=g0_bhhn[:, 0:1], scale=0.0)
            h_T = gru_elem(rz0, gin0, hn0, None, "p0")

            h_transposes(h_T, 0, "p0", want_sbuf=False)
            nc.gpsimd.collective_compute(
                "AllGather", OP.bypass, replica_groups=RG,
                ins=[hown[0].opt()], outs=[hfull[0].opt()],
            )

            # ---------------- conv depths ----------------
            h_nm = None
            for d in range(DEPTHS):
                aggT = pagg.tile([64, NPC], f32, name=f"aggT{d}", tag="agg")
                # root contribution first: start=True zeroes the accumulator
                for s in range(2):
                    nc.tensor.matmul(
                        aggT[0:64, s * 512:(s + 1) * 512],
                        rootw[d][:],
                        h_T[:, s * 512:(s + 1) * 512],
                        start=True, stop=False,
                    )

                # all gathers first (own-src ones lead: hown is ready
                # before the AllGather lands), so no GPSIMD multiply ever
                # blocks a queued gather or vice versa
                hsfs = {}
                for t in range(T):
                    hsf = hsfp.tile([P, DIM], f16, name=f"hsf{d}_{t}", tag="hsf")
                    src_dram = hown[d] if t < T_OWN else hfull[d]
                    nc.gpsimd.indirect_dma_start(
                        out=hsf[:], out_offset=None,
                        in_=src_dram[:, :],
                        in_offset=bass.IndirectOffsetOnAxis(ap=srcx[:, t:t + 1], axis=0),
                    )
                    hsfs[t] = hsf
                for t in range(T):
                    hsf = hsfs[t]
                    gps_tile = t >= T_OWN and (t % 8) in GPS_PAT
                    hv = hsf[:, :].rearrange("p (g l) -> p g l", l=8)
                    wsb = wsbp.tile([P, 4096], f16, name=f"wsb{d}_{t}", tag="wsb")
                    for q in range(4):
                        wps = pwe.tile([P, NPC], f32, name=f"we{d}_{t}_{q}", tag="pwe")
                        mm512(lambda o, n, _q=q, _t=t: wps[:, o:o + n],
                              hidT[d][:, t * P:(t + 1) * P],
                              lambda o, n, _q=q: w2p[d][:, _q * 1024 + o:_q * 1024 + o + n],
                              1024, True, True)
                        nc.scalar.activation(
                            wsb[:, q * 1024:(q + 1) * 1024], wps[:], AF.Copy
                        )
                        if b2bc is not None:
                            nc.vector.tensor_tensor(
                                out=wsb[:, q * 1024:(q + 1) * 1024],
                                in0=wsb[:, q * 1024:(q + 1) * 1024],
                                in1=b2bc[d][:, q * 1024:(q + 1) * 1024], op=OP.add,
                            )
                    eng = nc.gpsimd if gps_tile else nc.vector
                    eng.tensor_tensor(
                        out=wsb[:].rearrange("p (g o l) -> p g o l", o=64, l=8),
                        in0=wsb[:].rearrange("p (g o l) -> p g o l", o=64, l=8),
                        in1=hv[:, :, :].unsqueeze(2).to_broadcast([P, 8, 64, 8]),
                        op=OP.mult,
                    )
                    # fold-adds over the i_hi bits are contiguous-slice
                    # in-place adds (w2p column layout is (i_hi3, o, i_lo3));
                    # in-place keeps the op at two SBUF streams like the mult
                    for w in (2048, 1024, 512):
                        nc.vector.tensor_tensor(
                            out=wsb[:, 0:w], in0=wsb[:, 0:w], in1=wsb[:, w:2 * w],
                            op=OP.add,
                        )
                    msg = ep.tile([P, DIM], f16, name=f"msg{d}_{t}", tag="msg")
                    nc.vector.tensor_reduce(
                        out=msg[:], in_=wsb[:, 0:512].rearrange("p (o l) -> p o l", l=8),
                        axis=mybir.AxisListType.X, op=OP.add,
                    )
                    for s in range(2):
                        nc.tensor.matmul(
                            aggT[0:64, s * 512:(s + 1) * 512],
                            msg[:],
                            S[:, t * NPC + s * 512: t * NPC + (s + 1) * 512],
                            start=False, stop=(t == T - 1),
                        )
                xc = wp.tile([64, NPC], f16, name=f"xc{d}", tag="xc")
                nc.scalar.activation(xc[:], aggT[0:64, :], AF.Relu, bias=convb[d][:, 0:1])

                # ---- GRU(xc, h): r/z gates accumulate wih@x + whh@h in PSUM
                rz_ps = pwe.tile([P, NPC], f32, name=f"rz{d}", tag="pwe")
                for s in range(2):
                    nc.tensor.matmul(
                        rz_ps[0:128, s * 512:(s + 1) * 512], wihT[d][:, 0:128],
                        xc[:, s * 512:(s + 1) * 512], start=True, stop=False,
                    )
                    nc.tensor.matmul(
                        rz_ps[0:128, s * 512:(s + 1) * 512], whhT[d][:, 0:128],
                        h_T[:, s * 512:(s + 1) * 512], start=False, stop=True,
                    )
                rz = wp.tile([P, NPC], f16, name=f"rzs{d}", tag="gru_rz")
                nc.scalar.activation(rz[:], rz_ps[0:128, :], AF.Sigmoid, bias=brz[d][:, 0:1])

                gin_ps = pwe.tile([P, NPC], f32, name=f"gin{d}", tag="pwe")
                mm512(lambda o, n: gin_ps[0:64, o:o + n], wihT[d][:, 128:192],
                      lambda o, n: xc[:, o:o + n], NPC, True, True)
                gin = wp.tile([64, NPC], f16, name=f"gins{d}", tag="gru_gin")
                nc.scalar.activation(gin[:], gin_ps[0:64, :], AF.Identity, bias=bihn[d][:, 0:1])

                ghn_ps = pwe.tile([P, NPC], f32, name=f"ghn{d}", tag="pwe")
                mm512(lambda o, n: ghn_ps[0:64, o:o + n], whhT[d][:, 128:192],
                      lambda o, n: h_T[:, o:o + n], NPC, True, True)
                hn = wp.tile([64, NPC], f16, name=f"hns{d}", tag="gru_hn")
                nc.scalar.activation(hn[:], ghn_ps[0:64, :], AF.Identity, bias=bhhn[d][:, 0:1])
                h_T = gru_elem(rz, gin, hn, h_T, f"d{d}")

                if d < DEPTHS - 1:
                    h_transposes(h_T, d + 1, f"d{d}", want_sbuf=False)
                    nc.gpsimd.collective_compute(
                        "AllGather", OP.bypass, replica_groups=RG,
                        ins=[hown[d + 1].opt()], outs=[hfull[d + 1].opt()],
                    )
                else:
                    h_nm = h_transposes(h_T, None, f"d{d}", want_sbuf=True)
                    pooled_ps = pagg.tile([64, N_GRAPHS], f32, name="pooled_ps", tag="agg")
                    for c in range(8):
                        nc.tensor.matmul(
                            pooled_ps[0:64, :],
                            h_nm[c][:],
                            pS[:, c * N_GRAPHS:(c + 1) * N_GRAPHS],
                            start=(c == 0), stop=(c == 7),
                        )
                    pooled_sb = wp.tile([64, N_GRAPHS], f32, name="pooled_sb")
                    nc.scalar.activation(pooled_sb[:], pooled_ps[0:64, :], AF.Copy)
                    nc.sync.dma_start(ar_in[:, :], pooled_sb[:])

            # ---------------- pooling AllReduce + output MLP ----------------
            nc.gpsimd.collective_compute(
                "AllReduce", OP.add, replica_groups=RG,
                ins=[ar_in.opt()], outs=[ar_out.opt()],
            )
            pooled = wp.tile([64, N_GRAPHS], f32, name="pooled")
            nc.sync.dma_start(pooled[:], ar_out[:, :])
            pooled16 = wp.tile([64, N_GRAPHS], f16, name="pooled16")
            nc.scalar.activation(pooled16[:], pooled[:], AF.Copy)

            m1_ps = pagg.tile([64, N_GRAPHS], f32, name="m1_ps", tag="agg")
            nc.tensor.matmul(m1_ps[0:64, :], o0wT[:], pooled16[:], start=True, stop=True)
            m1r = wp.tile([64, N_GRAPHS], f16, name="m1r")
            nc.scalar.activation(m1r[:], m1_ps[0:64, :], AF.Relu, bias=o0b[:, 0:1])

            m2_ps = pagg.tile([64, N_GRAPHS], f32, name="m2_ps", tag="agg")
            nc.tensor.matmul(m2_ps[0:32, :], o1wT[:], m1r[:], start=True, stop=True)
            m2b = wp.tile([32, N_GRAPHS], f16, name="m2b")
            nc.scalar.activation(m2b[:], m2_ps[0:32, :], AF.Identity, bias=o1b[:, 0:1])

            m3_ps = pagg.tile([64, N_GRAPHS], f32, name="m3_ps", tag="agg")
            nc.tensor.matmul(m3_ps[0:1, :], o2wT[:], m2b[:], start=True, stop=True)
            ysb = wp.tile([1, N_GRAPHS], f32, name="ysb")
            nc.scalar.activation(ysb[:], m3_ps[0:1, :], AF.Identity, bias=o2b[:, 0:1])
            nc.sync.dma_start(y_d[:, :], ysb[:])

    nc.finalize()
    return nc


def _prep(inputs):
    """Host-side sharding + weight permutation. Returns (T, T_OWN, b2_zero, in_maps)."""
    g = lambda k: np.asarray(inputs[k])
    x = g("x").astype(np.float32)
    ea = g("edge_attr").astype(np.float32)
    ei = g("edge_index").astype(np.int64)
    batch = g("batch").astype(np.int64)
    src, dst = ei[0], ei[1]

    owner = dst // NPC
    core_ids = [np.nonzero(owner == c)[0] for c in range(NC)]

    # own-src edges (src owned by the same core) are ordered first; they can
    # gather h from the core-local copy before the AllGather completes.
    own_lists, gen_lists = [], []
    for c in range(NC):
        ids = core_ids[c]
        is_own = (src[ids] // NPC) == c
        own_lists.append(ids[is_own])
        gen_lists.append(ids[~is_own])
    T_OWN = max(1, min(len(o) for o in own_lists) // P)
    n_own_slots = T_OWN * P

    seqs = []
    for c in range(NC):
        own, gen = own_lists[c], gen_lists[c]
        own_used = own[:n_own_slots]
        spill = own[n_own_slots:]
        gen_all = np.concatenate([spill, gen])
        seqs.append((own_used, gen_all))
    T_GEN = max((len(gl) + P - 1) // P for _, gl in seqs)
    T = T_OWN + T_GEN
    EP = T * P

    cnt = np.bincount(batch, minlength=N_GRAPHS).astype(np.float32)
    inv = 1.0 / np.maximum(cnt, 1.0)

    mlp2_b = g("mlp2_b").astype(np.float32)
    b2_zero = bool(np.all(mlp2_b == 0))

    # ---- shared weights
    shared = {
        "fc0_wT": g("fc0_w").astype(np.float16).T.copy(),
        "fc0_b": g("fc0_b").astype(np.float32)[:, None],
        "g0_wihT": g("gru0_wih").astype(np.float16).T.copy(),
        "g0_brz": (g("gru0_bih") + g("gru0_bhh")).astype(np.float32)[:128, None],
        "g0_bihn": g("gru0_bih").astype(np.float32)[128:, None],
        "g0_bhhn": g("gru0_bhh").astype(np.float32)[128:, None],
        "o0wT": g("out0_w").astype(np.float16).T.copy(),
        "o0b": g("out0_b").astype(np.float32)[:, None],
        "o1wT": g("out1_w").astype(np.float16).T.copy(),
        "o1b": g("out1_b").astype(np.float32)[:, None],
        "o2wT": g("out2_w").astype(np.float16).T.copy(),
        "o2b": g("out2_b").astype(np.float32)[:, None],
    }
    mlp1_w = g("mlp1_w").astype(np.float32)
    mlp1_b = g("mlp1_b").astype(np.float32)
    mlp2_w = g("mlp2_w").astype(np.float32)
    root_w = g("root_w").astype(np.float32)
    conv_b = g("conv_b").astype(np.float32)
    gru_wih = g("gru_wih").astype(np.float32)
    gru_whh = g("gru_whh").astype(np.float32)
    gru_bih = g("gru_bih").astype(np.float32)
    gru_bhh = g("gru_bhh").astype(np.float32)
    for d in range(DEPTHS):
        # column layout (i_hi3, o, i_lo3): fold-adds over i become
        # contiguous-slice adds (DVE 2x mode needs packed operands)
        shared[f"w2p{d}"] = (
            mlp2_w[d].reshape(8, 8, 64, 128).transpose(3, 0, 2, 1).reshape(128, 4096)
        ).astype(np.float16)
        shared[f"m1wT{d}"] = mlp1_w[d].T.astype(np.float16).copy()
        shared[f"m1b{d}"] = mlp1_b[d][:, None].copy()
        shared[f"root{d}"] = root_w[d].astype(np.float16).copy()
        shared[f"convb{d}"] = conv_b[d][:, None].copy()
        shared[f"wihT{d}"] = gru_wih[d].T.astype(np.float16).copy()
        shared[f"whhT{d}"] = gru_whh[d].T.astype(np.float16).copy()
        shared[f"brz{d}"] = (gru_bih[d] + gru_bhh[d])[:128, None].copy()
        shared[f"bihn{d}"] = gru_bih[d][128:, None].copy()
        shared[f"bhhn{d}"] = gru_bhh[d][128:, None].copy()
        if not b2_zero:
            b2p = mlp2_b[d].reshape(8, 8, 64).transpose(0, 2, 1).reshape(4096)
            shared[f"b2bc{d}"] = np.broadcast_to(
                b2p.astype(np.float16), (P, 4096)
            ).copy()

    in_maps = []
    for c in range(NC):
        own_used, gen_all = seqs[c]
        ids = np.concatenate([own_used, np.full(n_own_slots - len(own_used), -1),
                              gen_all, np.full(EP - n_own_slots - len(gen_all), -1)])
        valid = ids >= 0
        idv = ids.copy()
        idv[~valid] = 0  # placeholder edge (zero S column kills contribution)
        src_pad = src[idv].astype(np.int32)
        src_pad[~valid] = c * NPC  # any in-range node
        # own-src tiles use LOCAL indices into hown
        src_pad[:n_own_slots] -= c * NPC
        ea_pad = ea[idv].astype(np.float32)
        ea_pad[~valid] = 0.0
        S_full = np.zeros((EP, NPC), np.float16)
        rows = np.nonzero(valid)[0]
        S_full[rows, dst[idv[rows]] - c * NPC] = 1.0
        S_tab = np.zeros((P, T * NPC), np.float16)
        for t in range(T):
            S_tab[:, t * NPC:(t + 1) * NPC] = S_full[t * P:(t + 1) * P]
        pm = np.zeros((NPC, N_GRAPHS), np.float16)
        nb = batch[c * NPC:(c + 1) * NPC]
        pm[np.arange(NPC), nb] = inv[nb].astype(np.float16)
        m = {
            "xT": x[c * NPC:(c + 1) * NPC].T.astype(np.float16).copy(),
            "eaT": ea_pad.T.astype(np.float16).copy(),
            "srcidx": src_pad.reshape(T, P).T.copy(),
            "S": S_tab,
            "poolS": pm,
        }
        m.update(shared)
        in_maps.append(m)
    return T, T_OWN, b2_zero, in_maps


def kernel(**inputs) -> np.ndarray:
    global LAST_EXEC_NS, LAST_RESULTS
    T, T_OWN, b2_zero, in_maps = _prep(inputs)
    key = (T, T_OWN, b2_zero)
    if key not in _CACHE:
        _CACHE[key] = _build(T, T_OWN, b2_zero)
    nc = _CACHE[key]

    from concourse.bass_utils import run_bass_kernel_spmd

    if TRACE:
        res = run_bass_kernel_spmd(
            nc, in_maps, list(range(NC)), trace=True, trace_cores=list(range(NC))
        )
        LAST_EXEC_NS = res.exec_time_ns
        LAST_RESULTS = res
    else:
        res = run_bass_kernel_spmd(nc, in_maps, list(range(NC)))
    return res.results[0]["y"].reshape(N_GRAPHS).astype(np.float32)


# revision 13
# speedup vs baseline: 1.3287x; 1.3287x over previous
"""NNConv+GRU message-passing network (ConvGRU) on 8 Trainium2 NeuronCores.

Strategy (v2, tuned from trace analysis of the v1 baseline):
  - Edges sharded by OWNER OF DST node (8 node ranges of 1024); scatter-add
    realized as matmul against a 0/1 selection matrix (exact dup handling).
  - h node-sharded for the GRU; AllGathered (fp16) once per conv layer.
    Edges whose SRC is also core-local are ordered first and gather h from
    the local copy, giving DVE work during the AllGather latency.
  - Per-edge weights We: PE computes hid@w2p into PSUM fp32 (fp16 inputs),
    ACT evacuates to one fp16 [128,4096] SBUF tile per edge-tile, then the
    per-edge matvec is: one broadcast multiply (DVE 2x mode, or GPSIMD for
    a subset of tiles to balance engines), three in-place strided fold-adds
    (DVE 2x), and one short tensor_reduce -> fp16 msg.
  - Everything on the h path is fp16 (fp32 matmuls cost 4 cyc/col vs 1).
  - GRU r/z: the wih@x and whh@h matmuls accumulate into one PSUM tile.

Self-contained: only needs numpy + the concourse/bass stack installed in the
container. All shapes hardcoded for this problem size.
"""
import numpy as np

DIM = 64
DEPTHS = 3
N_NODES = 8192
N_EDGES = 16384
N_GRAPHS = 64
NC = 8
NPC = N_NODES // NC   # 1024 nodes per core
P = 128

TRACE = False
LAST_EXEC_NS = None
LAST_RESULTS = None

_CACHE = {}

# tiles t (past the own-src block) with t % 8 in this set do their broadcast
# multiply on GPSIMD instead of DVE (engine balancing)
GPS_PAT = ()
SCAT_LAG = 3


def _build(T, T_OWN, b2_zero):
    """Build the (shared) 8-core SPMD program. Per-core data arrives via inputs."""
    import concourse.mybir as mybir
    import concourse.tile as tile
    from concourse import bacc
    import concourse.bass as bass
    from concourse.masks import make_identity

    f32 = mybir.dt.float32
    f16 = mybir.dt.float16
    i32 = mybir.dt.int32
    AF = mybir.ActivationFunctionType
    OP = mybir.AluOpType
    EP = T * P  # padded edge count per core

    nc = bacc.Bacc("TRN2", target_bir_lowering=False, debug=False, num_devices=NC)

    def din(name, shape, dt=f32):
        return nc.dram_tensor(name, shape, dt, kind="ExternalInput")

    xT_d = din("xT", [40, NPC], f16)
    eaT_d = din("eaT", [10, EP], f16)
    srcx_d = din("srcidx", [P, T], i32)
    S_d = din("S", [P, T * NPC], f16)
    pS_d = din("poolS", [NPC, N_GRAPHS], f16)
    fc0_wT_d = din("fc0_wT", [40, 32], f16)
    fc0_b_d = din("fc0_b", [32, 1])
    g0_wihT_d = din("g0_wihT", [32, 192], f16)
    g0_brz_d = din("g0_brz", [128, 1])
    g0_bihn_d = din("g0_bihn", [64, 1])
    g0_bhhn_d = din("g0_bhhn", [64, 1])
    w2p_d = [din(f"w2p{d}", [128, 4096], f16) for d in range(DEPTHS)]
    m1wT_d = [din(f"m1wT{d}", [10, 128], f16) for d in range(DEPTHS)]
    m1b_d = [din(f"m1b{d}", [128, 1]) for d in range(DEPTHS)]
    root_d = [din(f"root{d}", [64, 64], f16) for d in range(DEPTHS)]
    convb_d = [din(f"convb{d}", [64, 1]) for d in range(DEPTHS)]
    wihT_d = [din(f"wihT{d}", [64, 192], f16) for d in range(DEPTHS)]
    whhT_d = [din(f"whhT{d}", [64, 192], f16) for d in range(DEPTHS)]
    brz_d = [din(f"brz{d}", [128, 1]) for d in range(DEPTHS)]
    bihn_d = [din(f"bihn{d}", [64, 1]) for d in range(DEPTHS)]
    bhhn_d = [din(f"bhhn{d}", [64, 1]) for d in range(DEPTHS)]
    b2bc_d = None if b2_zero else [din(f"b2bc{d}", [128, 4096], f16) for d in range(DEPTHS)]
    o0wT_d = din("o0wT", [64, 64], f16)
    o0b_d = din("o0b", [64, 1])
    o1wT_d = din("o1wT", [64, 32], f16)
    o1b_d = din("o1b", [32, 1])
    o2wT_d = din("o2wT", [32, 1], f16)
    o2b_d = din("o2b", [1, 1])

    y_d = nc.dram_tensor("y", [1, N_GRAPHS], f32, kind="ExternalOutput")

    RG = [list(range(NC))]

    with nc.allow_low_precision("fp16 pipeline; final tolerance is 2e-2"), \
         tile.TileContext(nc) as tc:
        with (
            tc.tile_pool(name="const", bufs=1) as cp,
            tc.tile_pool(name="work", bufs=2) as wp,
            tc.tile_pool(name="wsbp", bufs=4) as wsbp,
            tc.tile_pool(name="edge", bufs=6) as ep,
            tc.tile_pool(name="hsfp", bufs=T + 3) as hsfp,
            tc.tile_pool(name="pwe", bufs=2, space="PSUM") as pwe,
            tc.tile_pool(name="pagg", bufs=1, space="PSUM") as pagg,
            tc.tile_pool(name="ptp", bufs=2, space="PSUM") as ptp,
            tc.tile_pool(name="dram", bufs=1, space="DRAM") as dp,
        ):
            # ---------------- constants to SBUF ----------------
            def load(name, dram, shape, dt=f32, ap=None):
                t = cp.tile(shape, dt, name=name)
                nc.sync.dma_start(t[:], dram[:, :] if ap is None else ap)
                return t

            # load order = DMA issue order: phase0 + edge-MLP inputs first so
            # compute starts while the big S / w2p / GRU tensors stream in.
            xT = load("xT_s", xT_d, [40, NPC], f16)
            fc0_wT = load("fc0_wT_s", fc0_wT_d, [40, 32], f16)
            fc0_b = load("fc0_b_s", fc0_b_d, [32, 1])
            g0_wihT = load("g0_wihT_s", g0_wihT_d, [32, 192], f16)
            g0_brz = load("g0_brz_s", g0_brz_d, [128, 1])
            g0_bihn = load("g0_bihn_s", g0_bihn_d, [64, 1])
            g0_bhhn = load("g0_bhhn_s", g0_bhhn_d, [64, 1])
            eaT = load("eaT_s", eaT_d, [10, EP], f16)
            m1wT = [load(f"m1wT_s{d}", m1wT_d[d], [10, 128], f16) for d in range(DEPTHS)]
            m1b = [load(f"m1b_s{d}", m1b_d[d], [128, 1]) for d in range(DEPTHS)]
            srcx = load("srcx_s", srcx_d, [P, T], i32)
            w2p = [load(f"w2p_s{d}", w2p_d[d], [128, 4096], f16) for d in range(DEPTHS)]
            S = cp.tile([P, T * NPC], f16, name="S_s")
            for t in range(T):
                nc.sync.dma_start(
                    S[:, t * NPC:(t + 1) * NPC], S_d[:, t * NPC:(t + 1) * NPC]
                )
            pS = cp.tile([P, 8 * N_GRAPHS], f16, name="pS_s")
            for c in range(8):
                nc.sync.dma_start(
                    pS[:, c * N_GRAPHS:(c + 1) * N_GRAPHS],
                    pS_d[c * P:(c + 1) * P, :],
                )
            rootw = [load(f"root_s{d}", root_d[d], [64, 64], f16) for d in range(DEPTHS)]
            convb = [load(f"convb_s{d}", convb_d[d], [64, 1]) for d in range(DEPTHS)]
            wihT = [load(f"wihT_s{d}", wihT_d[d], [64, 192], f16) for d in range(DEPTHS)]
            whhT = [load(f"whhT_s{d}", whhT_d[d], [64, 192], f16) for d in range(DEPTHS)]
            brz = [load(f"brz_s{d}", brz_d[d], [128, 1]) for d in range(DEPTHS)]
            bihn = [load(f"bihn_s{d}", bihn_d[d], [64, 1]) for d in range(DEPTHS)]
            bhhn = [load(f"bhhn_s{d}", bhhn_d[d], [64, 1]) for d in range(DEPTHS)]
            b2bc = (
                None if b2_zero else
                [load(f"b2bc_s{d}", b2bc_d[d], [128, 4096], f16) for d in range(DEPTHS)]
            )
            o0wT = load("o0wT_s", o0wT_d, [64, 64], f16)
            o0b = load("o0b_s", o0b_d, [64, 1])
            o1wT = load("o1wT_s", o1wT_d, [64, 32], f16)
            o1b = load("o1b_s", o1b_d, [32, 1])
            o2wT = load("o2wT_s", o2wT_d, [32, 1], f16)
            o2b = load("o2b_s", o2b_d, [1, 1])

            ident = cp.tile([64, 64], f16, name="ident")
            make_identity(nc, ident[:])

            hown = [dp.tile([NPC, DIM], f16, name=f"hown{d}") for d in range(DEPTHS)]
            hfull = [dp.tile([N_NODES, DIM], f16, name=f"hfull{d}") for d in range(DEPTHS)]
            ar_in = dp.tile([DIM, N_GRAPHS], f32, name="ar_in")
            ar_out = dp.tile([DIM, N_GRAPHS], f32, name="ar_out")

            # ---------------- helpers ----------------
            def mm512(out_ap_fn, lhsT, rhs_fn, n_total, start, stop):
                """matmuls in 512-wide chunks: out[:, s] = lhsT.T @ rhs[:, s]."""
                off = 0
                while off < n_total:
                    n = min(512, n_total - off)
                    nc.tensor.matmul(
                        out_ap_fn(off, n), lhsT, rhs_fn(off, n),
                        start=start, stop=stop,
                    )
                    off += n

            def gru_elem(rz_s, gi_n_s, hn_s, h_prev, tagp):
                """rz_s [128,1024] f16 (r||z post-sigmoid), gi_n_s/hn_s [64,1024] f16.
                Returns new h_T [64,1024] f16: h' = n + z*(h - n)."""
                # DVE needs equal base partitions for SBUF+SBUF tensor_tensor,
                # so shift the z half down to a base-0 tile via SBUF->SBUF DMA.
                z_s = wp.tile([64, NPC], f16, name=f"z_{tagp}", tag="gru_z")
                nc.sync.dma_start(z_s[:], rz_s[64:128, :])
                t1 = wp.tile([64, NPC], f16, name=f"t1_{tagp}", tag="gru_t1")
                nc.vector.tensor_tensor(out=t1[:], in0=rz_s[0:64, :], in1=hn_s[:], op=OP.mult)
                nc.vector.tensor_tensor(out=t1[:], in0=t1[:], in1=gi_n_s[:], op=OP.add)
                nt = wp.tile([64, NPC], f16, name=f"nt_{tagp}", tag="gru_nt")
                nc.scalar.activation(nt[:], t1[:], AF.Tanh)
                hm = wp.tile([64, NPC], f16, name=f"hm_{tagp}", tag="gru_hm")
                if h_prev is None:
                    # h=0: h' = n - z*n
                    nc.vector.tensor_tensor(out=hm[:], in0=z_s[:], in1=nt[:], op=OP.mult)
                    hnew = wp.tile([64, NPC], f16, name=f"h_{tagp}", tag="hT")
                    nc.vector.tensor_tensor(out=hnew[:], in0=nt[:], in1=hm[:], op=OP.subtract)
                else:
                    nc.vector.tensor_tensor(out=hm[:], in0=h_prev[:], in1=nt[:], op=OP.subtract)
                    nc.vector.tensor_tensor(out=hm[:], in0=hm[:], in1=z_s[:], op=OP.mult)
                    hnew = wp.tile([64, NPC], f16, name=f"h_{tagp}", tag="hT")
                    nc.vector.tensor_tensor(out=hnew[:], in0=hm[:], in1=nt[:], op=OP.add)
                return hnew

            def h_transposes(h_T, d_out, tagp, want_sbuf):
                """PE-transpose h_T [64,1024] f16 -> 8 [128,64] node-major SBUF
                tiles (PSUM can't feed DMA directly), DMA each to hown[d_out]."""
                sb = []
                for c in range(8):
                    tp = ptp.tile([P, DIM], f16, name=f"tp_{tagp}_{c}", tag="tp")
                    nc.tensor.transpose(
                        out=tp[:], in_=h_T[:, c * P:(c + 1) * P], identity=ident[:]
                    )
                    hm = wp.tile([P, DIM], f16, name=f"hnm_{tagp}_{c}", tag=f"hnm{c}")
                    if c % 2 == 0:
                        nc.scalar.activation(hm[:], tp[:], AF.Copy)
                    else:
                        nc.vector.tensor_copy(hm[:], tp[:])
                    if d_out is not None:
                        nc.sync.dma_start(hown[d_out][c * P:(c + 1) * P, :], hm[:])
                    if want_sbuf:
                        sb.append(hm)
                return sb

            # ---------------- edge-MLP hidden states, all depths upfront ----
            hidT = []
            for d in range(DEPTHS):
                ht = cp.tile([P, EP], f16, name=f"hidT{d}")
                off = 0
                while off < EP:
                    n = min(1024, EP - off)
                    hp = pwe.tile([P, NPC], f32, name=f"hid_ps{d}_{off}", tag="pwe")
                    mm512(lambda o, nn, _b=off: hp[:, o:o + nn], m1wT[d][:],
                          lambda o, nn, _b=off: eaT[:, _b + o:_b + o + nn], n, True, True)
                    nc.scalar.activation(
                        ht[:, off:off + n], hp[:, 0:n], AF.Relu, bias=m1b[d][:, 0:1]
                    )
                    off += n
                hidT.append(ht)

            # ---------------- phase 0: fc0 + gru0 (h0 = 0) ----------------
            x0_ps = pwe.tile([P, NPC], f32, name="x0_ps", tag="pwe")
            mm512(lambda o, n: x0_ps[0:32, o:o + n], fc0_wT[:],
                  lambda o, n: xT[:, o:o + n], NPC, True, True)
            x0r = wp.tile([32, NPC], f16, name="x0r")
            nc.scalar.activation(x0r[:], x0_ps[0:32, :], AF.Relu, bias=fc0_b[:, 0:1])

            g0rz_ps = pwe.tile([P, NPC], f32, name="g0rz_ps", tag="pwe")
            mm512(lambda o, n: g0rz_ps[0:128, o:o + n], g0_wihT[:, 0:128],
                  lambda o, n: x0r[:, o:o + n], NPC, True, True)
            rz0 = wp.tile([P, NPC], f16, name="rz0", tag="gru_rz")
            nc.scalar.activation(rz0[:], g0rz_ps[0:128, :], AF.Sigmoid, bias=g0_brz[:, 0:1])

            g0n_ps = pwe.tile([P, NPC], f32, name="g0n_ps", tag="pwe")
            mm512(lambda o, n: g0n_ps[0:64, o:o + n], g0_wihT[:, 128:192],
                  lambda o, n: x0r[:, o:o + n], NPC, True, True)
            gin0 = wp.tile([64, NPC], f16, name="gin0", tag="gru_gin")
            nc.scalar.activation(gin0[:], g0n_ps[0:64, :], AF.Identity, bias=g0_bihn[:, 0:1])
            # h=0 so gh_n = bhh_n: broadcast bhh_n across columns (scale=0 trick)
            hn0 = wp.tile([64, NPC], f16, name="hn0", tag="gru_hn")
            nc.scalar.activation(hn0[:], gin0[:], AF.Identity, bias=g0_bhhn[:, 0:1], scale=0.0)
            h_T = gru_elem(rz0, gin0, hn0, None, "p0")

            h_transposes(h_T, 0, "p0", want_sbuf=False)
            nc.gpsimd.collective_compute(
                "AllGather", OP.bypass, replica_groups=RG,
                ins=[hown[0].opt()], outs=[hfull[0].opt()],
            )

            # ---------------- conv depths ----------------
            h_nm = None
            for d in range(DEPTHS):
                aggT = pagg.tile([64, NPC], f32, name=f"aggT{d}", tag="agg")
                # root contribution first: start=True zeroes the accumulator
                for s in range(2):
                    nc.tensor.matmul(
                        aggT[0:64, s * 512:(s + 1) * 512],
                        rootw[d][:],
                        h_T[:, s * 512:(s + 1) * 512],
                        start=True, stop=False,
                    )

                # all gathers first (own-src ones lead: hown is ready
                # before the AllGather lands), so no GPSIMD multiply ever
                # blocks a queued gather or vice versa
                hsfs = {}
                for t in range(T):
                    hsf = hsfp.tile([P, DIM], f16, name=f"hsf{d}_{t}", tag="hsf")
                    src_dram = hown[d] if t < T_OWN else hfull[d]
                    nc.gpsimd.indirect_dma_start(
                        out=hsf[:], out_offset=None,
                        in_=src_dram[:, :],
                        in_offset=bass.IndirectOffsetOnAxis(ap=srcx[:, t:t + 1], axis=0),
                    )
                    hsfs[t] = hsf
                def emit_scatter(t):
                    for s in range(2):
                        nc.tensor.matmul(
                            aggT[0:64, s * 512:(s + 1) * 512],
                            msgs[t][:],
                            S[:, t * NPC + s * 512: t * NPC + (s + 1) * 512],
                            start=False, stop=(t == T - 1),
                        )

                msgs = {}
                for t in range(T):
                    hsf = hsfs[t]
                    hv = hsf[:, :].rearrange("p (g l) -> p g l", l=8)
                    wsb = wsbp.tile([P, 4096], f16, name=f"wsb{d}_{t}", tag="wsb")
                    for q in range(4):
                        wps = pwe.tile([P, NPC], f32, name=f"we{d}_{t}_{q}", tag="pwe")
                        mm512(lambda o, n, _q=q, _t=t: wps[:, o:o + n],
                              hidT[d][:, t * P:(t + 1) * P],
                              lambda o, n, _q=q: w2p[d][:, _q * 1024 + o:_q * 1024 + o + n],
                              1024, True, True)
                        nc.scalar.activation(
                            wsb[:, q * 1024:(q + 1) * 1024], wps[:], AF.Copy
                        )
                        if b2bc is not None:
                            nc.vector.tensor_tensor(
                                out=wsb[:, q * 1024:(q + 1) * 1024],
                                in0=wsb[:, q * 1024:(q + 1) * 1024],
                                in1=b2bc[d][:, q * 1024:(q + 1) * 1024], op=OP.add,
                            )
                    nc.vector.tensor_tensor(
                        out=wsb[:].rearrange("p (g o l) -> p g o l", o=64, l=8),
                        in0=wsb[:].rearrange("p (g o l) -> p g o l", o=64, l=8),
                        in1=hv[:, :, :].unsqueeze(2).to_broadcast([P, 8, 64, 8]),
                        op=OP.mult,
                    )
                    # fold-adds over the i_hi bits are contiguous-slice
                    # in-place adds (w2p column layout is (i_hi3, o, i_lo3))
                    for w in (2048, 1024, 512):
                        nc.vector.tensor_tensor(
                            out=wsb[:, 0:w], in0=wsb[:, 0:w], in1=wsb[:, w:2 * w],
                            op=OP.add,
                        )
                    msg = ep.tile([P, DIM], f16, name=f"msg{d}_{t}", tag="msg")
                    nc.vector.tensor_reduce(
                        out=msg[:], in_=wsb[:, 0:512].rearrange("p (o l) -> p o l", l=8),
                        axis=mybir.AxisListType.X, op=OP.add,
                    )
                    msgs[t] = msg
                    # scatter matmuls trail by SCAT_LAG tiles so one late msg
                    # never head-of-line-blocks the PE queue's We matmuls
                    if t >= SCAT_LAG:
                        emit_scatter(t - SCAT_LAG)
                for t in range(max(0, T - SCAT_LAG), T):
                    emit_scatter(t)
                xc = wp.tile([64, NPC], f16, name=f"xc{d}", tag="xc")
                nc.scalar.activation(xc[:], aggT[0:64, :], AF.Relu, bias=convb[d][:, 0:1])

                # ---- GRU(xc, h): r/z gates accumulate wih@x + whh@h in PSUM
                rz_ps = pwe.tile([P, NPC], f32, name=f"rz{d}", tag="pwe")
                for s in range(2):
                    nc.tensor.matmul(
                        rz_ps[0:128, s * 512:(s + 1) * 512], wihT[d][:, 0:128],
                        xc[:, s * 512:(s + 1) * 512], start=True, stop=False,
                    )
                    nc.tensor.matmul(
                        rz_ps[0:128, s * 512:(s + 1) * 512], whhT[d][:, 0:128],
                        h_T[:, s * 512:(s + 1) * 512], start=False, stop=True,
                    )
                rz = wp.tile([P, NPC], f16, name=f"rzs{d}", tag="gru_rz")
                nc.scalar.activation(rz[:], rz_ps[0:128, :], AF.Sigmoid, bias=brz[d][:, 0:1])

                gin_ps = pwe.tile([P, NPC], f32, name=f"gin{d}", tag="pwe")
                mm512(lambda o, n: gin_ps[0:64, o:o + n], wihT[d][:, 128:192],
                      lambda o, n: xc[:, o:o + n], NPC, True, True)
                gin = wp.tile([64, NPC], f16, name=f"gins{d}", tag="gru_gin")
                nc.scalar.activation(gin[:], gin_ps[0:64, :], AF.Identity, bias=bihn[d][:, 0:1])

                ghn_ps = pwe.tile([P, NPC], f32, name=f"ghn{d}", tag="pwe")
                mm512(lambda o, n: ghn_ps[0:64, o:o + n], whhT[d][:, 128:192],
                      lambda o, n: h_T[:, o:o + n], NPC, True, True)
                hn = wp.tile([64, NPC], f16, name=f"hns{d}", tag="gru_hn")
                nc.scalar.activation(hn[:], ghn_ps[0:64, :], AF.Identity, bias=bhhn[d][:, 0:1])
                h_T = gru_elem(rz, gin, hn, h_T, f"d{d}")

                if d < DEPTHS - 1:
                    h_transposes(h_T, d + 1, f"d{d}", want_sbuf=False)
                    nc.gpsimd.collective_compute(
                        "AllGather", OP.bypass, replica_groups=RG,
                        ins=[hown[d + 1].opt()], outs=[hfull[d + 1].opt()],
                    )
                else:
                    h_nm = h_transposes(h_T, None, f"d{d}", want_sbuf=True)
                    pooled_ps = pagg.tile([64, N_GRAPHS], f32, name="pooled_ps", tag="agg")
                    for c in range(8):
                        nc.tensor.matmul(
                            pooled_ps[0:64, :],
                            h_nm[c][:],
                            pS[:, c * N_GRAPHS:(c + 1) * N_GRAPHS],
                            start=(c == 0), stop=(c == 7),
                        )
                    pooled_sb = wp.tile([64, N_GRAPHS], f32, name="pooled_sb")
                    nc.scalar.activation(pooled_sb[:], pooled_ps[0:64, :], AF.Copy)
                    nc.sync.dma_start(ar_in[:, :], pooled_sb[:])

            # ---------------- pooling AllReduce + output MLP ----------------
            nc.gpsimd.collective_compute(
                "AllReduce", OP.add, replica_groups=RG,
                ins=[ar_in.opt()], outs=[ar_out.opt()],
            )
            pooled = wp.tile([64, N_GRAPHS], f32, name="pooled")
            nc.sync.dma_start(pooled[:], ar_out[:, :])
            pooled16 = wp.tile([64, N_GRAPHS], f16, name="pooled16")
            nc.scalar.activation(pooled16[:], pooled[:], AF.Copy)

            m1_ps = pagg.tile([64, N_GRAPHS], f32, name="m1_ps", tag="agg")
            nc.tensor.matmul(m1_ps[0:64, :], o0wT[:], pooled16[:], start=True, stop=True)
            m1r = wp.tile([64, N_GRAPHS], f16, name="m1r")
            nc.scalar.activation(m1r[:], m1_ps[0:64, :], AF.Relu, bias=o0b[:, 0:1])

            m2_ps = pagg.tile([64, N_GRAPHS], f32, name="m2_ps", tag="agg")
            nc.tensor.matmul(m2_ps[0:32, :], o1wT[:], m1r[:], start=True, stop=True)
            m2b = wp.tile([32, N_GRAPHS], f16, name="m2b")
            nc.scalar.activation(m2b[:], m2_ps[0:32, :], AF.Identity, bias=o1b[:, 0:1])

            m3_ps = pagg.tile([64, N_GRAPHS], f32, name="m3_ps", tag="agg")
            nc.tensor.matmul(m3_ps[0:1, :], o2wT[:], m2b[:], start=True, stop=True)
            ysb = wp.tile([1, N_GRAPHS], f32, name="ysb")
            nc.scalar.activation(ysb[:], m3_ps[0:1, :], AF.Identity, bias=o2b[:, 0:1])
            nc.sync.dma_start(y_d[:, :], ysb[:])

    nc.finalize()
    return nc


def _prep(inputs):
    """Host-side sharding + weight permutation. Returns (T, T_OWN, b2_zero, in_maps)."""
    g = lambda k: np.asarray(inputs[k])
    x = g("x").astype(np.float32)
    ea = g("edge_attr").astype(np.float32)
    ei = g("edge_index").astype(np.int64)
    batch = g("batch").astype(np.int64)
    src, dst = ei[0], ei[1]

    owner = dst // NPC
    core_ids = [np.nonzero(owner == c)[0] for c in range(NC)]

    # own-src edges (src owned by the same core) are ordered first; they can
    # gather h from the core-local copy before the AllGather completes.
    own_lists, gen_lists = [], []
    for c in range(NC):
        ids = core_ids[c]
        is_own = (src[ids] // NPC) == c
        own_lists.append(ids[is_own])
        gen_lists.append(ids[~is_own])
    T_OWN = max(1, min(len(o) for o in own_lists) // P)
    n_own_slots = T_OWN * P

    seqs = []
    for c in range(NC):
        own, gen = own_lists[c], gen_lists[c]
        own_used = own[:n_own_slots]
        spill = own[n_own_slots:]
        gen_all = np.concatenate([spill, gen])
        seqs.append((own_used, gen_all))
    T_GEN = max((len(gl) + P - 1) // P for _, gl in seqs)
    T = T_OWN + T_GEN
    EP = T * P

    cnt = np.bincount(batch, minlength=N_GRAPHS).astype(np.float32)
    inv = 1.0 / np.maximum(cnt, 1.0)

    mlp2_b = g("mlp2_b").astype(np.float32)
    b2_zero = bool(np.all(mlp2_b == 0))

    # ---- shared weights
    shared = {
        "fc0_wT": g("fc0_w").astype(np.float16).T.copy(),
        "fc0_b": g("fc0_b").astype(np.float32)[:, None],
        "g0_wihT": g("gru0_wih").astype(np.float16).T.copy(),
        "g0_brz": (g("gru0_bih") + g("gru0_bhh")).astype(np.float32)[:128, None],
        "g0_bihn": g("gru0_bih").astype(np.float32)[128:, None],
        "g0_bhhn": g("gru0_bhh").astype(np.float32)[128:, None],
        "o0wT": g("out0_w").astype(np.float16).T.copy(),
        "o0b": g("out0_b").astype(np.float32)[:, None],
        "o1wT": g("out1_w").astype(np.float16).T.copy(),
        "o1b": g("out1_b").astype(np.float32)[:, None],
        "o2wT": g("out2_w").astype(np.float16).T.copy(),
        "o2b": g("out2_b").astype(np.float32)[:, None],
    }
    mlp1_w = g("mlp1_w").astype(np.float32)
    mlp1_b = g("mlp1_b").astype(np.float32)
    mlp2_w = g("mlp2_w").astype(np.float32)
    root_w = g("root_w").astype(np.float32)
    conv_b = g("conv_b").astype(np.float32)
    gru_wih = g("gru_wih").astype(np.float32)
    gru_whh = g("gru_whh").astype(np.float32)
    gru_bih = g("gru_bih").astype(np.float32)
    gru_bhh = g("gru_bhh").astype(np.float32)
    for d in range(DEPTHS):
        # column layout (i_hi3, o, i_lo3): fold-adds over i become
        # contiguous-slice adds (DVE 2x mode needs packed operands)
        shared[f"w2p{d}"] = (
            mlp2_w[d].reshape(8, 8, 64, 128).transpose(3, 0, 2, 1).reshape(128, 4096)
        ).astype(np.float16)
        shared[f"m1wT{d}"] = mlp1_w[d].T.astype(np.float16).copy()
        shared[f"m1b{d}"] = mlp1_b[d][:, None].copy()
        shared[f"root{d}"] = root_w[d].astype(np.float16).copy()
        shared[f"convb{d}"] = conv_b[d][:, None].copy()
        shared[f"wihT{d}"] = gru_wih[d].T.astype(np.float16).copy()
        shared[f"whhT{d}"] = gru_whh[d].T.astype(np.float16).copy()
        shared[f"brz{d}"] = (gru_bih[d] + gru_bhh[d])[:128, None].copy()
        shared[f"bihn{d}"] = gru_bih[d][128:, None].copy()
        shared[f"bhhn{d}"] = gru_bhh[d][128:, None].copy()
        if not b2_zero:
            b2p = mlp2_b[d].reshape(8, 8, 64).transpose(0, 2, 1).reshape(4096)
            shared[f"b2bc{d}"] = np.broadcast_to(
                b2p.astype(np.float16), (P, 4096)
            ).copy()

    in_maps = []
    for c in range(NC):
        own_used, gen_all = seqs[c]
        ids = np.concatenate([own_used, np.full(n_own_slots - len(own_used), -1),
                              gen_all, np.full(EP - n_own_slots - len(gen_all), -1)])
        valid = ids >= 0
        idv = ids.copy()
        idv[~valid] = 0  # placeholder edge (zero S column kills contribution)
        src_pad = src[idv].astype(np.int32)
        src_pad[~valid] = c * NPC  # any in-range node
        # own-src tiles use LOCAL indices into hown
        src_pad[:n_own_slots] -= c * NPC
        ea_pad = ea[idv].astype(np.float32)
        ea_pad[~valid] = 0.0
        S_full = np.zeros((EP, NPC), np.float16)
        rows = np.nonzero(valid)[0]
        S_full[rows, dst[idv[rows]] - c * NPC] = 1.0
        S_tab = np.zeros((P, T * NPC), np.float16)
        for t in range(T):
            S_tab[:, t * NPC:(t + 1) * NPC] = S_full[t * P:(t + 1) * P]
        pm = np.zeros((NPC, N_GRAPHS), np.float16)
        nb = batch[c * NPC:(c + 1) * NPC]
        pm[np.arange(NPC), nb] = inv[nb].astype(np.float16)
        m = {
            "xT": x[c * NPC:(c + 1) * NPC].T.astype(np.float16).copy(),
            "eaT": ea_pad.T.astype(np.float16).copy(),
            "srcidx": src_pad.reshape(T, P).T.copy(),
            "S": S_tab,
            "poolS": pm,
        }
        m.update(shared)
        in_maps.append(m)
    return T, T_OWN, b2_zero, in_maps


def kernel(**inputs) -> np.ndarray:
    global LAST_EXEC_NS, LAST_RESULTS
    T, T_OWN, b2_zero, in_maps = _prep(inputs)
    key = (T, T_OWN, b2_zero)
    if key not in _CACHE:
        _CACHE[key] = _build(T, T_OWN, b2_zero)
    nc = _CACHE[key]

    from concourse.bass_utils import run_bass_kernel_spmd

    if TRACE:
        res = run_bass_kernel_spmd(
            nc, in_maps, list(range(NC)), trace=True, trace_cores=list(range(NC))
        )
        LAST_EXEC_NS = res.exec_time_ns
        LAST_RESULTS = res
    else:
        res = run_bass_kernel_spmd(nc, in_maps, list(range(NC)))
    return res.results[0]["y"].reshape(N_GRAPHS).astype(np.float32)


# revision 14
# speedup vs baseline: 1.4078x; 1.0595x over previous
"""NNConv+GRU message-passing network (ConvGRU) on 8 Trainium2 NeuronCores.

Strategy (v2, tuned from trace analysis of the v1 baseline):
  - Edges sharded by OWNER OF DST node (8 node ranges of 1024); scatter-add
    realized as matmul against a 0/1 selection matrix (exact dup handling).
  - h node-sharded for the GRU; AllGathered (fp16) once per conv layer.
    Edges whose SRC is also core-local are ordered first and gather h from
    the local copy, giving DVE work during the AllGather latency.
  - Per-edge weights We: PE computes hid@w2p into PSUM fp32 (fp16 inputs),
    ACT evacuates to one fp16 [128,4096] SBUF tile per edge-tile, then the
    per-edge matvec is: one broadcast multiply (DVE 2x mode, or GPSIMD for
    a subset of tiles to balance engines), three in-place strided fold-adds
    (DVE 2x), and one short tensor_reduce -> fp16 msg.
  - Everything on the h path is fp16 (fp32 matmuls cost 4 cyc/col vs 1).
  - GRU r/z: the wih@x and whh@h matmuls accumulate into one PSUM tile.

Self-contained: only needs numpy + the concourse/bass stack installed in the
container. All shapes hardcoded for this problem size.
"""
import numpy as np

DIM = 64
DEPTHS = 3
N_NODES = 8192
N_EDGES = 16384
N_GRAPHS = 64
NC = 8
NPC = N_NODES // NC   # 1024 nodes per core
P = 128

TRACE = False
LAST_EXEC_NS = None
LAST_RESULTS = None

_CACHE = {}

# tiles t (past the own-src block) with t % 8 in this set do their broadcast
# multiply on GPSIMD instead of DVE (engine balancing)
GPS_PAT = ()
SCAT_LAG = 3


def _build(T, T_OWN, b2_zero):
    """Build the (shared) 8-core SPMD program. Per-core data arrives via inputs."""
    import concourse.mybir as mybir
    import concourse.tile as tile
    from concourse import bacc
    import concourse.bass as bass
    from concourse.masks import make_identity

    f32 = mybir.dt.float32
    f16 = mybir.dt.float16
    i32 = mybir.dt.int32
    AF = mybir.ActivationFunctionType
    OP = mybir.AluOpType
    EP = T * P  # padded edge count per core

    nc = bacc.Bacc("TRN2", target_bir_lowering=False, debug=False, num_devices=NC)

    def din(name, shape, dt=f32):
        return nc.dram_tensor(name, shape, dt, kind="ExternalInput")

    xT_d = din("xT", [40, NPC], f16)
    eaT_d = din("eaT", [10, EP], f16)
    srcx_d = din("srcidx", [P, T], i32)
    S_d = din("S", [P, T * NPC], f16)
    pS_d = din("poolS", [NPC, N_GRAPHS], f16)
    fc0_wT_d = din("fc0_wT", [40, 32], f16)
    fc0_b_d = din("fc0_b", [32, 1])
    g0_wihT_d = din("g0_wihT", [32, 192], f16)
    g0_brz_d = din("g0_brz", [128, 1])
    g0_bihn_d = din("g0_bihn", [64, 1])
    g0_bhhn_d = din("g0_bhhn", [64, 1])
    w2p_d = [din(f"w2p{d}", [128, 4096], f16) for d in range(DEPTHS)]
    m1wT_d = [din(f"m1wT{d}", [10, 128], f16) for d in range(DEPTHS)]
    m1b_d = [din(f"m1b{d}", [128, 1]) for d in range(DEPTHS)]
    root_d = [din(f"root{d}", [64, 64], f16) for d in range(DEPTHS)]
    convb_d = [din(f"convb{d}", [64, 1]) for d in range(DEPTHS)]
    wihT_d = [din(f"wihT{d}", [64, 192], f16) for d in range(DEPTHS)]
    whhT_d = [din(f"whhT{d}", [64, 192], f16) for d in range(DEPTHS)]
    brz_d = [din(f"brz{d}", [128, 1]) for d in range(DEPTHS)]
    bihn_d = [din(f"bihn{d}", [64, 1]) for d in range(DEPTHS)]
    bhhn_d = [din(f"bhhn{d}", [64, 1]) for d in range(DEPTHS)]
    b2bc_d = None if b2_zero else [din(f"b2bc{d}", [128, 4096], f16) for d in range(DEPTHS)]
    o0wT_d = din("o0wT", [64, 64], f16)
    o0b_d = din("o0b", [64, 1])
    o1wT_d = din("o1wT", [64, 32], f16)
    o1b_d = din("o1b", [32, 1])
    o2wT_d = din("o2wT", [32, 1], f16)
    o2b_d = din("o2b", [1, 1])

    y_d = nc.dram_tensor("y", [1, N_GRAPHS], f32, kind="ExternalOutput")

    RG = [list(range(NC))]

    with nc.allow_low_precision("fp16 pipeline; final tolerance is 2e-2"), \
         tile.TileContext(nc) as tc:
        with (
            tc.tile_pool(name="const", bufs=1) as cp,
            tc.tile_pool(name="work", bufs=2) as wp,
            tc.tile_pool(name="wsbp", bufs=6) as wsbp,
            tc.tile_pool(name="edge", bufs=6) as ep,
            tc.tile_pool(name="hsfp", bufs=T + 3) as hsfp,
            tc.tile_pool(name="pwe", bufs=2, space="PSUM") as pwe,
            tc.tile_pool(name="pagg", bufs=1, space="PSUM") as pagg,
            tc.tile_pool(name="ptp", bufs=2, space="PSUM") as ptp,
            tc.tile_pool(name="dram", bufs=1, space="DRAM") as dp,
        ):
            # ---------------- constants to SBUF ----------------
            def load(name, dram, shape, dt=f32, ap=None):
                t = cp.tile(shape, dt, name=name)
                nc.sync.dma_start(t[:], dram[:, :] if ap is None else ap)
                return t

            # load order = DMA issue order: phase0 + edge-MLP inputs first so
            # compute starts while the big S / w2p / GRU tensors stream in.
            xT = load("xT_s", xT_d, [40, NPC], f16)
            fc0_wT = load("fc0_wT_s", fc0_wT_d, [40, 32], f16)
            fc0_b = load("fc0_b_s", fc0_b_d, [32, 1])
            g0_wihT = load("g0_wihT_s", g0_wihT_d, [32, 192], f16)
            g0_brz = load("g0_brz_s", g0_brz_d, [128, 1])
            g0_bihn = load("g0_bihn_s", g0_bihn_d, [64, 1])
            g0_bhhn = load("g0_bhhn_s", g0_bhhn_d, [64, 1])
            eaT = load("eaT_s", eaT_d, [10, EP], f16)
            m1wT = [load(f"m1wT_s{d}", m1wT_d[d], [10, 128], f16) for d in range(DEPTHS)]
            m1b = [load(f"m1b_s{d}", m1b_d[d], [128, 1]) for d in range(DEPTHS)]
            srcx = load("srcx_s", srcx_d, [P, T], i32)
            w2p = [load(f"w2p_s{d}", w2p_d[d], [128, 4096], f16) for d in range(DEPTHS)]
            S = cp.tile([P, T * NPC], f16, name="S_s")
            for t in range(T):
                nc.sync.dma_start(
                    S[:, t * NPC:(t + 1) * NPC], S_d[:, t * NPC:(t + 1) * NPC]
                )
            pS = cp.tile([P, 8 * N_GRAPHS], f16, name="pS_s")
            for c in range(8):
                nc.sync.dma_start(
                    pS[:, c * N_GRAPHS:(c + 1) * N_GRAPHS],
                    pS_d[c * P:(c + 1) * P, :],
                )
            rootw = [load(f"root_s{d}", root_d[d], [64, 64], f16) for d in range(DEPTHS)]
            convb = [load(f"convb_s{d}", convb_d[d], [64, 1]) for d in range(DEPTHS)]
            wihT = [load(f"wihT_s{d}", wihT_d[d], [64, 192], f16) for d in range(DEPTHS)]
            whhT = [load(f"whhT_s{d}", whhT_d[d], [64, 192], f16) for d in range(DEPTHS)]
            brz = [load(f"brz_s{d}", brz_d[d], [128, 1]) for d in range(DEPTHS)]
            bihn = [load(f"bihn_s{d}", bihn_d[d], [64, 1]) for d in range(DEPTHS)]
            bhhn = [load(f"bhhn_s{d}", bhhn_d[d], [64, 1]) for d in range(DEPTHS)]
            b2bc = (
                None if b2_zero else
                [load(f"b2bc_s{d}", b2bc_d[d], [128, 4096], f16) for d in range(DEPTHS)]
            )
            o0wT = load("o0wT_s", o0wT_d, [64, 64], f16)
            o0b = load("o0b_s", o0b_d, [64, 1])
            o1wT = load("o1wT_s", o1wT_d, [64, 32], f16)
            o1b = load("o1b_s", o1b_d, [32, 1])
            o2wT = load("o2wT_s", o2wT_d, [32, 1], f16)
            o2b = load("o2b_s", o2b_d, [1, 1])

            ident = cp.tile([64, 64], f16, name="ident")
            make_identity(nc, ident[:])

            hown = [dp.tile([NPC, DIM], f16, name=f"hown{d}") for d in range(DEPTHS)]
            hfull = [dp.tile([N_NODES, DIM], f16, name=f"hfull{d}") for d in range(DEPTHS)]
            ar_in = dp.tile([DIM, N_GRAPHS], f32, name="ar_in")
            ar_out = dp.tile([DIM, N_GRAPHS], f32, name="ar_out")

            # ---------------- helpers ----------------
            def mm512(out_ap_fn, lhsT, rhs_fn, n_total, start, stop):
                """matmuls in 512-wide chunks: out[:, s] = lhsT.T @ rhs[:, s]."""
                off = 0
                while off < n_total:
                    n = min(512, n_total - off)
                    nc.tensor.matmul(
                        out_ap_fn(off, n), lhsT, rhs_fn(off, n),
                        start=start, stop=stop,
                    )
                    off += n

            def gru_elem(rz_s, gi_n_s, hn_s, h_prev, tagp):
                """rz_s [128,1024] f16 (r||z post-sigmoid), gi_n_s/hn_s [64,1024] f16.
                Returns new h_T [64,1024] f16: h' = n + z*(h - n)."""
                # DVE needs equal base partitions for SBUF+SBUF tensor_tensor,
                # so shift the z half down to a base-0 tile via SBUF->SBUF DMA.
                z_s = wp.tile([64, NPC], f16, name=f"z_{tagp}", tag="gru_z")
                nc.sync.dma_start(z_s[:], rz_s[64:128, :])
                t1 = wp.tile([64, NPC], f16, name=f"t1_{tagp}", tag="gru_t1")
                nc.vector.tensor_tensor(out=t1[:], in0=rz_s[0:64, :], in1=hn_s[:], op=OP.mult)
                nc.vector.tensor_tensor(out=t1[:], in0=t1[:], in1=gi_n_s[:], op=OP.add)
                nt = wp.tile([64, NPC], f16, name=f"nt_{tagp}", tag="gru_nt")
                nc.scalar.activation(nt[:], t1[:], AF.Tanh)
                hm = wp.tile([64, NPC], f16, name=f"hm_{tagp}", tag="gru_hm")
                if h_prev is None:
                    # h=0: h' = n - z*n
                    nc.vector.tensor_tensor(out=hm[:], in0=z_s[:], in1=nt[:], op=OP.mult)
                    hnew = wp.tile([64, NPC], f16, name=f"h_{tagp}", tag="hT")
                    nc.vector.tensor_tensor(out=hnew[:], in0=nt[:], in1=hm[:], op=OP.subtract)
                else:
                    nc.vector.tensor_tensor(out=hm[:], in0=h_prev[:], in1=nt[:], op=OP.subtract)
                    nc.vector.tensor_tensor(out=hm[:], in0=hm[:], in1=z_s[:], op=OP.mult)
                    hnew = wp.tile([64, NPC], f16, name=f"h_{tagp}", tag="hT")
                    nc.vector.tensor_tensor(out=hnew[:], in0=hm[:], in1=nt[:], op=OP.add)
                return hnew

            def h_transposes(h_T, d_out, tagp, want_sbuf):
                """PE-transpose h_T [64,1024] f16 -> 8 [128,64] node-major SBUF
                tiles (PSUM can't feed DMA directly), DMA each to hown[d_out]."""
                sb = []
                for c in range(8):
                    tp = ptp.tile([P, DIM], f16, name=f"tp_{tagp}_{c}", tag="tp")
                    nc.tensor.transpose(
                        out=tp[:], in_=h_T[:, c * P:(c + 1) * P], identity=ident[:]
                    )
                    hm = wp.tile([P, DIM], f16, name=f"hnm_{tagp}_{c}", tag=f"hnm{c}")
                    if c % 2 == 0:
                        nc.scalar.activation(hm[:], tp[:], AF.Copy)
                    else:
                        nc.vector.tensor_copy(hm[:], tp[:])
                    if d_out is not None:
                        nc.sync.dma_start(hown[d_out][c * P:(c + 1) * P, :], hm[:])
                    if want_sbuf:
                        sb.append(hm)
                return sb

            # ---------------- edge-MLP hidden states, all depths upfront ----
            hidT = []
            for d in range(DEPTHS):
                ht = cp.tile([P, EP], f16, name=f"hidT{d}")
                off = 0
                while off < EP:
                    n = min(1024, EP - off)
                    hp = pwe.tile([P, NPC], f32, name=f"hid_ps{d}_{off}", tag="pwe")
                    mm512(lambda o, nn, _b=off: hp[:, o:o + nn], m1wT[d][:],
                          lambda o, nn, _b=off: eaT[:, _b + o:_b + o + nn], n, True, True)
                    nc.scalar.activation(
                        ht[:, off:off + n], hp[:, 0:n], AF.Relu, bias=m1b[d][:, 0:1]
                    )
                    off += n
                hidT.append(ht)

            # ---------------- phase 0: fc0 + gru0 (h0 = 0) ----------------
            x0_ps = pwe.tile([P, NPC], f32, name="x0_ps", tag="pwe")
            mm512(lambda o, n: x0_ps[0:32, o:o + n], fc0_wT[:],
                  lambda o, n: xT[:, o:o + n], NPC, True, True)
            x0r = wp.tile([32, NPC], f16, name="x0r")
            nc.scalar.activation(x0r[:], x0_ps[0:32, :], AF.Relu, bias=fc0_b[:, 0:1])

            g0rz_ps = pwe.tile([P, NPC], f32, name="g0rz_ps", tag="pwe")
            mm512(lambda o, n: g0rz_ps[0:128, o:o + n], g0_wihT[:, 0:128],
                  lambda o, n: x0r[:, o:o + n], NPC, True, True)
            rz0 = wp.tile([P, NPC], f16, name="rz0", tag="gru_rz")
            nc.scalar.activation(rz0[:], g0rz_ps[0:128, :], AF.Sigmoid, bias=g0_brz[:, 0:1])

            g0n_ps = pwe.tile([P, NPC], f32, name="g0n_ps", tag="pwe")
            mm512(lambda o, n: g0n_ps[0:64, o:o + n], g0_wihT[:, 128:192],
                  lambda o, n: x0r[:, o:o + n], NPC, True, True)
            gin0 = wp.tile([64, NPC], f16, name="gin0", tag="gru_gin")
            nc.scalar.activation(gin0[:], g0n_ps[0:64, :], AF.Identity, bias=g0_bihn[:, 0:1])
            # h=0 so gh_n = bhh_n: broadcast bhh_n across columns (scale=0 trick)
            hn0 = wp.tile([64, NPC], f16, name="hn0", tag="gru_hn")
            nc.scalar.activation(hn0[:], gin0[:], AF.Identity, bias=g0_bhhn[:, 0:1], scale=0.0)
            h_T = gru_elem(rz0, gin0, hn0, None, "p0")

            h_transposes(h_T, 0, "p0", want_sbuf=False)
            nc.gpsimd.collective_compute(
                "AllGather", OP.bypass, replica_groups=RG,
                ins=[hown[0].opt()], outs=[hfull[0].opt()],
            )

            # ---------------- conv depths ----------------
            h_nm = None
            for d in range(DEPTHS):
                aggT = pagg.tile([64, NPC], f32, name=f"aggT{d}", tag="agg")
                # root contribution first: start=True zeroes the accumulator
                for s in range(2):
                    nc.tensor.matmul(
                        aggT[0:64, s * 512:(s + 1) * 512],
                        rootw[d][:],
                        h_T[:, s * 512:(s + 1) * 512],
                        start=True, stop=False,
                    )

                # all gathers first (own-src ones lead: hown is ready
                # before the AllGather lands), so no GPSIMD multiply ever
                # blocks a queued gather or vice versa
                hsfs = {}
                for t in range(T):
                    hsf = hsfp.tile([P, DIM], f16, name=f"hsf{d}_{t}", tag="hsf")
                    src_dram = hown[d] if t < T_OWN else hfull[d]
                    nc.gpsimd.indirect_dma_start(
                        out=hsf[:], out_offset=None,
                        in_=src_dram[:, :],
                        in_offset=bass.IndirectOffsetOnAxis(ap=srcx[:, t:t + 1], axis=0),
                    )
                    hsfs[t] = hsf
                def emit_scatter(t):
                    for s in range(2):
                        nc.tensor.matmul(
                            aggT[0:64, s * 512:(s + 1) * 512],
                            msgs[t][:],
                            S[:, t * NPC + s * 512: t * NPC + (s + 1) * 512],
                            start=False, stop=(t == T - 1),
                        )

                msgs = {}
                for t in range(T):
                    hsf = hsfs[t]
                    hv = hsf[:, :].rearrange("p (g l) -> p g l", l=8)
                    wsb = wsbp.tile([P, 4096], f16, name=f"wsb{d}_{t}", tag="wsb")
                    for q in range(4):
                        wps = pwe.tile([P, NPC], f32, name=f"we{d}_{t}_{q}", tag="pwe")
                        mm512(lambda o, n, _q=q, _t=t: wps[:, o:o + n],
                              hidT[d][:, t * P:(t + 1) * P],
                              lambda o, n, _q=q: w2p[d][:, _q * 1024 + o:_q * 1024 + o + n],
                              1024, True, True)
                        nc.scalar.activation(
                            wsb[:, q * 1024:(q + 1) * 1024], wps[:], AF.Copy
                        )
                        if b2bc is not None:
                            nc.vector.tensor_tensor(
                                out=wsb[:, q * 1024:(q + 1) * 1024],
                                in0=wsb[:, q * 1024:(q + 1) * 1024],
                                in1=b2bc[d][:, q * 1024:(q + 1) * 1024], op=OP.add,
                            )
                    nc.vector.tensor_tensor(
                        out=wsb[:].rearrange("p (g o l) -> p g o l", o=64, l=8),
                        in0=wsb[:].rearrange("p (g o l) -> p g o l", o=64, l=8),
                        in1=hv[:, :, :].unsqueeze(2).to_broadcast([P, 8, 64, 8]),
                        op=OP.mult,
                    )
                    # fold-adds over the i_hi bits are contiguous-slice
                    # in-place adds (w2p column layout is (i_hi3, o, i_lo3))
                    for w in (2048, 1024, 512):
                        nc.vector.tensor_tensor(
                            out=wsb[:, 0:w], in0=wsb[:, 0:w], in1=wsb[:, w:2 * w],
                            op=OP.add,
                        )
                    msg = ep.tile([P, DIM], f16, name=f"msg{d}_{t}", tag="msg")
                    nc.vector.tensor_reduce(
                        out=msg[:], in_=wsb[:, 0:512].rearrange("p (o l) -> p o l", l=8),
                        axis=mybir.AxisListType.X, op=OP.add,
                    )
                    msgs[t] = msg
                    # scatter matmuls trail by SCAT_LAG tiles so one late msg
                    # never head-of-line-blocks the PE queue's We matmuls
                    if t >= SCAT_LAG:
                        emit_scatter(t - SCAT_LAG)
                for t in range(max(0, T - SCAT_LAG), T):
                    emit_scatter(t)
                xc = wp.tile([64, NPC], f16, name=f"xc{d}", tag="xc")
                nc.scalar.activation(xc[:], aggT[0:64, :], AF.Relu, bias=convb[d][:, 0:1])

                # ---- GRU(xc, h): r/z gates accumulate wih@x + whh@h in PSUM
                rz_ps = pwe.tile([P, NPC], f32, name=f"rz{d}", tag="pwe")
                for s in range(2):
                    nc.tensor.matmul(
                        rz_ps[0:128, s * 512:(s + 1) * 512], wihT[d][:, 0:128],
                        xc[:, s * 512:(s + 1) * 512], start=True, stop=False,
                    )
                    nc.tensor.matmul(
                        rz_ps[0:128, s * 512:(s + 1) * 512], whhT[d][:, 0:128],
                        h_T[:, s * 512:(s + 1) * 512], start=False, stop=True,
                    )
                rz = wp.tile([P, NPC], f16, name=f"rzs{d}", tag="gru_rz")
                nc.scalar.activation(rz[:], rz_ps[0:128, :], AF.Sigmoid, bias=brz[d][:, 0:1])

                gin_ps = pwe.tile([P, NPC], f32, name=f"gin{d}", tag="pwe")
                mm512(lambda o, n: gin_ps[0:64, o:o + n], wihT[d][:, 128:192],
                      lambda o, n: xc[:, o:o + n], NPC, True, True)
                gin = wp.tile([64, NPC], f16, name=f"gins{d}", tag="gru_gin")
                nc.scalar.activation(gin[:], gin_ps[0:64, :], AF.Identity, bias=bihn[d][:, 0:1])

                ghn_ps = pwe.tile([P, NPC], f32, name=f"ghn{d}", tag="pwe")
                mm512(lambda o, n: ghn_ps[0:64, o:o + n], whhT[d][:, 128:192],
                      lambda o, n: h_T[:, o:o + n], NPC, True, True)
                hn = wp.tile([64, NPC], f16, name=f"hns{d}", tag="gru_hn")
                nc.scalar.activation(hn[:], ghn_ps[0:64, :], AF.Identity, bias=bhhn[d][:, 0:1])
                h_T = gru_elem(rz, gin, hn, h_T, f"d{d}")

                if d < DEPTHS - 1:
                    h_transposes(h_T, d + 1, f"d{d}", want_sbuf=False)
                    nc.gpsimd.collective_compute(
                        "AllGather", OP.bypass, replica_groups=RG,
                        ins=[hown[d + 1].opt()], outs=[hfull[d + 1].opt()],
                    )
                else:
                    h_nm = h_transposes(h_T, None, f"d{d}", want_sbuf=True)
                    pooled_ps = pagg.tile([64, N_GRAPHS], f32, name="pooled_ps", tag="agg")
                    for c in range(8):
                        nc.tensor.matmul(
                            pooled_ps[0:64, :],
                            h_nm[c][:],
                            pS[:, c * N_GRAPHS:(c + 1) * N_GRAPHS],
                            start=(c == 0), stop=(c == 7),
                        )
                    pooled_sb = wp.tile([64, N_GRAPHS], f32, name="pooled_sb")
                    nc.scalar.activation(pooled_sb[:], pooled_ps[0:64, :], AF.Copy)
                    nc.sync.dma_start(ar_in[:, :], pooled_sb[:])

            # ---------------- pooling AllReduce + output MLP ----------------
            nc.gpsimd.collective_compute(
                "AllReduce", OP.add, replica_groups=RG,
                ins=[ar_in.opt()], outs=[ar_out.opt()],
            )
            pooled = wp.tile([64, N_GRAPHS], f32, name="pooled")
            nc.sync.dma_start(pooled[:], ar_out[:, :])
            pooled16 = wp.tile([64, N_GRAPHS], f16, name="pooled16")
            nc.scalar.activation(pooled16[:], pooled[:], AF.Copy)

            m1_ps = pagg.tile([64, N_GRAPHS], f32, name="m1_ps", tag="agg")
            nc.tensor.matmul(m1_ps[0:64, :], o0wT[:], pooled16[:], start=True, stop=True)
            m1r = wp.tile([64, N_GRAPHS], f16, name="m1r")
            nc.scalar.activation(m1r[:], m1_ps[0:64, :], AF.Relu, bias=o0b[:, 0:1])

            m2_ps = pagg.tile([64, N_GRAPHS], f32, name="m2_ps", tag="agg")
            nc.tensor.matmul(m2_ps[0:32, :], o1wT[:], m1r[:], start=True, stop=True)
            m2b = wp.tile([32, N_GRAPHS], f16, name="m2b")
            nc.scalar.activation(m2b[:], m2_ps[0:32, :], AF.Identity, bias=o1b[:, 0:1])

            m3_ps = pagg.tile([64, N_GRAPHS], f32, name="m3_ps", tag="agg")
            nc.tensor.matmul(m3_ps[0:1, :], o2wT[:], m2b[:], start=True, stop=True)
            ysb = wp.tile([1, N_GRAPHS], f32, name="ysb")
            nc.scalar.activation(ysb[:], m3_ps[0:1, :], AF.Identity, bias=o2b[:, 0:1])
            nc.sync.dma_start(y_d[:, :], ysb[:])

    nc.finalize()
    return nc


def _prep(inputs):
    """Host-side sharding + weight permutation. Returns (T, T_OWN, b2_zero, in_maps)."""
    g = lambda k: np.asarray(inputs[k])
    x = g("x").astype(np.float32)
    ea = g("edge_attr").astype(np.float32)
    ei = g("edge_index").astype(np.int64)
    batch = g("batch").astype(np.int64)
    src, dst = ei[0], ei[1]

    owner = dst // NPC
    core_ids = [np.nonzero(owner == c)[0] for c in range(NC)]

    # own-src edges (src owned by the same core) are ordered first; they can
    # gather h from the core-local copy before the AllGather completes.
    own_lists, gen_lists = [], []
    for c in range(NC):
        ids = core_ids[c]
        is_own = (src[ids] // NPC) == c
        own_lists.append(ids[is_own])
        gen_lists.append(ids[~is_own])
    min_own = min(len(o) for o in own_lists)
    T_OWN = max(1, min(2, (min_own + P - 1) // P))
    n_own_slots = T_OWN * P

    seqs = []
    for c in range(NC):
        own, gen = own_lists[c], gen_lists[c]
        own_used = own[:n_own_slots]
        spill = own[n_own_slots:]
        gen_all = np.concatenate([spill, gen])
        seqs.append((own_used, gen_all))
    T_GEN = max((len(gl) + P - 1) // P for _, gl in seqs)
    T = T_OWN + T_GEN
    EP = T * P

    cnt = np.bincount(batch, minlength=N_GRAPHS).astype(np.float32)
    inv = 1.0 / np.maximum(cnt, 1.0)

    mlp2_b = g("mlp2_b").astype(np.float32)
    b2_zero = bool(np.all(mlp2_b == 0))

    # ---- shared weights
    shared = {
        "fc0_wT": g("fc0_w").astype(np.float16).T.copy(),
        "fc0_b": g("fc0_b").astype(np.float32)[:, None],
        "g0_wihT": g("gru0_wih").astype(np.float16).T.copy(),
        "g0_brz": (g("gru0_bih") + g("gru0_bhh")).astype(np.float32)[:128, None],
        "g0_bihn": g("gru0_bih").astype(np.float32)[128:, None],
        "g0_bhhn": g("gru0_bhh").astype(np.float32)[128:, None],
        "o0wT": g("out0_w").astype(np.float16).T.copy(),
        "o0b": g("out0_b").astype(np.float32)[:, None],
        "o1wT": g("out1_w").astype(np.float16).T.copy(),
        "o1b": g("out1_b").astype(np.float32)[:, None],
        "o2wT": g("out2_w").astype(np.float16).T.copy(),
        "o2b": g("out2_b").astype(np.float32)[:, None],
    }
    mlp1_w = g("mlp1_w").astype(np.float32)
    mlp1_b = g("mlp1_b").astype(np.float32)
    mlp2_w = g("mlp2_w").astype(np.float32)
    root_w = g("root_w").astype(np.float32)
    conv_b = g("conv_b").astype(np.float32)
    gru_wih = g("gru_wih").astype(np.float32)
    gru_whh = g("gru_whh").astype(np.float32)
    gru_bih = g("gru_bih").astype(np.float32)
    gru_bhh = g("gru_bhh").astype(np.float32)
    for d in range(DEPTHS):
        # column layout (i_hi3, o, i_lo3): fold-adds over i become
        # contiguous-slice adds (DVE 2x mode needs packed operands)
        shared[f"w2p{d}"] = (
            mlp2_w[d].reshape(8, 8, 64, 128).transpose(3, 0, 2, 1).reshape(128, 4096)
        ).astype(np.float16)
        shared[f"m1wT{d}"] = mlp1_w[d].T.astype(np.float16).copy()
        shared[f"m1b{d}"] = mlp1_b[d][:, None].copy()
        shared[f"root{d}"] = root_w[d].astype(np.float16).copy()
        shared[f"convb{d}"] = conv_b[d][:, None].copy()
        shared[f"wihT{d}"] = gru_wih[d].T.astype(np.float16).copy()
        shared[f"whhT{d}"] = gru_whh[d].T.astype(np.float16).copy()
        shared[f"brz{d}"] = (gru_bih[d] + gru_bhh[d])[:128, None].copy()
        shared[f"bihn{d}"] = gru_bih[d][128:, None].copy()
        shared[f"bhhn{d}"] = gru_bhh[d][128:, None].copy()
        if not b2_zero:
            b2p = mlp2_b[d].reshape(8, 8, 64).transpose(0, 2, 1).reshape(4096)
            shared[f"b2bc{d}"] = np.broadcast_to(
                b2p.astype(np.float16), (P, 4096)
            ).copy()

    in_maps = []
    for c in range(NC):
        own_used, gen_all = seqs[c]
        ids = np.concatenate([own_used, np.full(n_own_slots - len(own_used), -1),
                              gen_all, np.full(EP - n_own_slots - len(gen_all), -1)])
        valid = ids >= 0
        idv = ids.copy()
        idv[~valid] = 0  # placeholder edge (zero S column kills contribution)
        src_pad = src[idv].astype(np.int32)
        src_pad[~valid] = c * NPC  # any in-range node
        # own-src tiles use LOCAL indices into hown
        src_pad[:n_own_slots] -= c * NPC
        ea_pad = ea[idv].astype(np.float32)
        ea_pad[~valid] = 0.0
        S_full = np.zeros((EP, NPC), np.float16)
        rows = np.nonzero(valid)[0]
        S_full[rows, dst[idv[rows]] - c * NPC] = 1.0
        S_tab = np.zeros((P, T * NPC), np.float16)
        for t in range(T):
            S_tab[:, t * NPC:(t + 1) * NPC] = S_full[t * P:(t + 1) * P]
        pm = np.zeros((NPC, N_GRAPHS), np.float16)
        nb = batch[c * NPC:(c + 1) * NPC]
        pm[np.arange(NPC), nb] = inv[nb].astype(np.float16)
        m = {
            "xT": x[c * NPC:(c + 1) * NPC].T.astype(np.float16).copy(),
            "eaT": ea_pad.T.astype(np.float16).copy(),
            "srcidx": src_pad.reshape(T, P).T.copy(),
            "S": S_tab,
            "poolS": pm,
        }
        m.update(shared)
        in_maps.append(m)
    return T, T_OWN, b2_zero, in_maps


def kernel(**inputs) -> np.ndarray:
    global LAST_EXEC_NS, LAST_RESULTS
    T, T_OWN, b2_zero, in_maps = _prep(inputs)
    key = (T, T_OWN, b2_zero)
    if key not in _CACHE:
        _CACHE[key] = _build(T, T_OWN, b2_zero)
    nc = _CACHE[key]

    from concourse.bass_utils import run_bass_kernel_spmd

    if TRACE:
        res = run_bass_kernel_spmd(
            nc, in_maps, list(range(NC)), trace=True, trace_cores=list(range(NC))
        )
        LAST_EXEC_NS = res.exec_time_ns
        LAST_RESULTS = res
    else:
        res = run_bass_kernel_spmd(nc, in_maps, list(range(NC)))
    return res.results[0]["y"].reshape(N_GRAPHS).astype(np.float32)


# revision 16
# speedup vs baseline: 1.4319x; 1.0171x over previous
"""NNConv+GRU message-passing network (ConvGRU) on 8 Trainium2 NeuronCores.

Strategy (v2, tuned from trace analysis of the v1 baseline):
  - Edges sharded by OWNER OF DST node (8 node ranges of 1024); scatter-add
    realized as matmul against a 0/1 selection matrix (exact dup handling).
  - h node-sharded for the GRU; AllGathered (fp16) once per conv layer.
    Edges whose SRC is also core-local are ordered first and gather h from
    the local copy, giving DVE work during the AllGather latency.
  - Per-edge weights We: PE computes hid@w2p into PSUM fp32 (fp16 inputs),
    ACT evacuates to one fp16 [128,4096] SBUF tile per edge-tile, then the
    per-edge matvec is: one broadcast multiply (DVE 2x mode, or GPSIMD for
    a subset of tiles to balance engines), three in-place strided fold-adds
    (DVE 2x), and one short tensor_reduce -> fp16 msg.
  - Everything on the h path is fp16 (fp32 matmuls cost 4 cyc/col vs 1).
  - GRU r/z: the wih@x and whh@h matmuls accumulate into one PSUM tile.

Self-contained: only needs numpy + the concourse/bass stack installed in the
container. All shapes hardcoded for this problem size.
"""
import numpy as np

DIM = 64
DEPTHS = 3
N_NODES = 8192
N_EDGES = 16384
N_GRAPHS = 64
NC = 8
NPC = N_NODES // NC   # 1024 nodes per core
P = 128

TRACE = False
LAST_EXEC_NS = None
LAST_RESULTS = None

_CACHE = {}

# tiles t (past the own-src block) with t % 8 in this set do their broadcast
# multiply on GPSIMD instead of DVE (engine balancing)
GPS_PAT = ()
SCAT_LAG = 3


def _build(T, T_OWN, b2_zero):
    """Build the (shared) 8-core SPMD program. Per-core data arrives via inputs."""
    import concourse.mybir as mybir
    import concourse.tile as tile
    from concourse import bacc
    import concourse.bass as bass
    from concourse.masks import make_identity

    f32 = mybir.dt.float32
    f16 = mybir.dt.float16
    i32 = mybir.dt.int32
    AF = mybir.ActivationFunctionType
    OP = mybir.AluOpType
    EP = T * P  # padded edge count per core

    nc = bacc.Bacc("TRN2", target_bir_lowering=False, debug=False, num_devices=NC)

    def din(name, shape, dt=f32):
        return nc.dram_tensor(name, shape, dt, kind="ExternalInput")

    xT_d = din("xT", [40, NPC], f16)
    eaT_d = din("eaT", [10, EP], f16)
    srcx_d = din("srcidx", [P, T], i32)
    S_d = din("S", [P, T * NPC], f16)
    pS_d = din("poolS", [P, 8 * N_GRAPHS], f16)
    # all small fp32 bias vectors as columns of one tensor (one DMA);
    # all small fp16 weight mats packed along columns of one tensor
    bp_d = din("biaspack", [P, 22])
    wp16_d = din("wpack16", [64, 2049], f16)
    w2p_d = [din(f"w2p{d}", [128, 4096], f16) for d in range(DEPTHS)]
    b2bc_d = None if b2_zero else [din(f"b2bc{d}", [128, 4096], f16) for d in range(DEPTHS)]

    y_d = nc.dram_tensor("y", [1, N_GRAPHS], f32, kind="ExternalOutput")

    RG = [list(range(NC))]

    with nc.allow_low_precision("fp16 pipeline; final tolerance is 2e-2"), \
         tile.TileContext(nc) as tc:
        with (
            tc.tile_pool(name="const", bufs=1) as cp,
            tc.tile_pool(name="work", bufs=2) as wp,
            tc.tile_pool(name="wsbp", bufs=6) as wsbp,
            tc.tile_pool(name="edge", bufs=6) as ep,
            tc.tile_pool(name="hsfp", bufs=T + 3) as hsfp,
            tc.tile_pool(name="pwe", bufs=2, space="PSUM") as pwe,
            tc.tile_pool(name="pagg", bufs=1, space="PSUM") as pagg,
            tc.tile_pool(name="ptp", bufs=2, space="PSUM") as ptp,
            tc.tile_pool(name="dram", bufs=1, space="DRAM") as dp,
        ):
            # ---------------- constants to SBUF ----------------
            def load(name, dram, shape, dt=f32, ap=None):
                t = cp.tile(shape, dt, name=name)
                nc.sync.dma_start(t[:], dram[:, :] if ap is None else ap)
                return t

            # load order = DMA issue order (Sync issues serially at
            # ~0.6-1us each, so everything small is packed into two DMAs)
            xT = load("xT_s", xT_d, [40, NPC], f16)
            bpk = load("bp_s", bp_d, [P, 22])
            wpk = load("wp16_s", wp16_d, [64, 2049], f16)
            eaT = load("eaT_s", eaT_d, [10, EP], f16)
            srcx = load("srcx_s", srcx_d, [P, T], i32)
            w2p = [load(f"w2p_s{d}", w2p_d[d], [128, 4096], f16) for d in range(DEPTHS)]
            S = load("S_s", S_d, [P, T * NPC], f16)
            pS = load("pS_s", pS_d, [P, 8 * N_GRAPHS], f16)
            b2bc = (
                None if b2_zero else
                [load(f"b2bc_s{d}", b2bc_d[d], [128, 4096], f16) for d in range(DEPTHS)]
            )

            bcol = [0]
            def bslice(rows):
                j = bcol[0]; bcol[0] += 1
                return bpk[0:rows, j:j + 1]
            fc0_b = bslice(32)
            g0_brz = bslice(128)
            g0_bihn = bslice(64)
            g0_bhhn = bslice(64)
            m1b = [bslice(128) for d in range(DEPTHS)]
            convb = [bslice(64) for d in range(DEPTHS)]
            brz = [bslice(128) for d in range(DEPTHS)]
            bihn = [bslice(64) for d in range(DEPTHS)]
            bhhn = [bslice(64) for d in range(DEPTHS)]
            o0b = bslice(64)
            o1b = bslice(32)
            o2b = bslice(1)

            wcol = [0]
            def wslice(rows, cols):
                j = wcol[0]; wcol[0] += cols
                return wpk[0:rows, j:j + cols]
            fc0_wT = wslice(40, 32)
            g0_wihT = wslice(32, 192)
            m1wT = [wslice(10, 128) for d in range(DEPTHS)]
            rootw = [wslice(64, 64) for d in range(DEPTHS)]
            wihT = [wslice(64, 192) for d in range(DEPTHS)]
            whhT = [wslice(64, 192) for d in range(DEPTHS)]
            o0wT = wslice(64, 64)
            o1wT = wslice(64, 32)
            o2wT = wslice(32, 1)

            ident = cp.tile([64, 64], f16, name="ident")
            make_identity(nc, ident[:])

            hown = [dp.tile([NPC, DIM], f16, name=f"hown{d}") for d in range(DEPTHS)]
            hfull = [dp.tile([N_NODES, DIM], f16, name=f"hfull{d}") for d in range(DEPTHS)]
            ar_in = dp.tile([DIM, N_GRAPHS], f32, name="ar_in")
            ar_out = dp.tile([DIM, N_GRAPHS], f32, name="ar_out")

            # ---------------- helpers ----------------
            def mm512(out_ap_fn, lhsT, rhs_fn, n_total, start, stop):
                """matmuls in 512-wide chunks: out[:, s] = lhsT.T @ rhs[:, s]."""
                off = 0
                while off < n_total:
                    n = min(512, n_total - off)
                    nc.tensor.matmul(
                        out_ap_fn(off, n), lhsT, rhs_fn(off, n),
                        start=start, stop=stop,
                    )
                    off += n

            def gru_elem(rz_s, gi_n_s, hn_s, h_prev, tagp):
                """rz_s [128,1024] f16 (r||z post-sigmoid), gi_n_s/hn_s [64,1024] f16.
                Returns new h_T [64,1024] f16: h' = n + z*(h - n)."""
                # DVE needs equal base partitions for SBUF+SBUF tensor_tensor,
                # so shift the z half down to a base-0 tile via SBUF->SBUF DMA.
                z_s = wp.tile([64, NPC], f16, name=f"z_{tagp}", tag="gru_z")
                nc.sync.dma_start(z_s[:], rz_s[64:128, :])
                t1 = wp.tile([64, NPC], f16, name=f"t1_{tagp}", tag="gru_t1")
                nc.vector.tensor_tensor(out=t1[:], in0=rz_s[0:64, :], in1=hn_s[:], op=OP.mult)
                nc.vector.tensor_tensor(out=t1[:], in0=t1[:], in1=gi_n_s[:], op=OP.add)
                nt = wp.tile([64, NPC], f16, name=f"nt_{tagp}", tag="gru_nt")
                nc.scalar.activation(nt[:], t1[:], AF.Tanh)
                hm = wp.tile([64, NPC], f16, name=f"hm_{tagp}", tag="gru_hm")
                if h_prev is None:
                    # h=0: h' = n - z*n
                    nc.vector.tensor_tensor(out=hm[:], in0=z_s[:], in1=nt[:], op=OP.mult)
                    hnew = wp.tile([64, NPC], f16, name=f"h_{tagp}", tag="hT")
                    nc.vector.tensor_tensor(out=hnew[:], in0=nt[:], in1=hm[:], op=OP.subtract)
                else:
                    nc.vector.tensor_tensor(out=hm[:], in0=h_prev[:], in1=nt[:], op=OP.subtract)
                    nc.vector.tensor_tensor(out=hm[:], in0=hm[:], in1=z_s[:], op=OP.mult)
                    hnew = wp.tile([64, NPC], f16, name=f"h_{tagp}", tag="hT")
                    nc.vector.tensor_tensor(out=hnew[:], in0=hm[:], in1=nt[:], op=OP.add)
                return hnew

            def h_transposes(h_T, d_out, tagp, want_sbuf):
                """PE-transpose h_T [64,1024] f16 -> 8 [128,64] node-major SBUF
                tiles (PSUM can't feed DMA directly), DMA each to hown[d_out]."""
                sb = []
                for c in range(8):
                    tp = ptp.tile([P, DIM], f16, name=f"tp_{tagp}_{c}", tag="tp")
                    nc.tensor.transpose(
                        out=tp[:], in_=h_T[:, c * P:(c + 1) * P], identity=ident[:]
                    )
                    hm = wp.tile([P, DIM], f16, name=f"hnm_{tagp}_{c}", tag=f"hnm{c}")
                    if c % 2 == 0:
                        nc.scalar.activation(hm[:], tp[:], AF.Copy)
                    else:
                        nc.vector.tensor_copy(hm[:], tp[:])
                    if d_out is not None:
                        nc.sync.dma_start(hown[d_out][c * P:(c + 1) * P, :], hm[:])
                    if want_sbuf:
                        sb.append(hm)
                return sb

            # ---------------- edge-MLP hidden states, all depths upfront ----
            hidT = []
            for d in range(DEPTHS):
                ht = cp.tile([P, EP], f16, name=f"hidT{d}")
                off = 0
                while off < EP:
                    n = min(1024, EP - off)
                    hp = pwe.tile([P, NPC], f32, name=f"hid_ps{d}_{off}", tag="pwe")
                    mm512(lambda o, nn, _b=off: hp[:, o:o + nn], m1wT[d],
                          lambda o, nn, _b=off: eaT[:, _b + o:_b + o + nn], n, True, True)
                    nc.scalar.activation(
                        ht[:, off:off + n], hp[:, 0:n], AF.Relu, bias=m1b[d]
                    )
                    off += n
                hidT.append(ht)

            # ---------------- phase 0: fc0 + gru0 (h0 = 0) ----------------
            x0_ps = pwe.tile([P, NPC], f32, name="x0_ps", tag="pwe")
            mm512(lambda o, n: x0_ps[0:32, o:o + n], fc0_wT,
                  lambda o, n: xT[:, o:o + n], NPC, True, True)
            x0r = wp.tile([32, NPC], f16, name="x0r")
            nc.scalar.activation(x0r[:], x0_ps[0:32, :], AF.Relu, bias=fc0_b)

            g0rz_ps = pwe.tile([P, NPC], f32, name="g0rz_ps", tag="pwe")
            mm512(lambda o, n: g0rz_ps[0:128, o:o + n], g0_wihT[:, 0:128],
                  lambda o, n: x0r[:, o:o + n], NPC, True, True)
            rz0 = wp.tile([P, NPC], f16, name="rz0", tag="gru_rz")
            nc.scalar.activation(rz0[:], g0rz_ps[0:128, :], AF.Sigmoid, bias=g0_brz)

            g0n_ps = pwe.tile([P, NPC], f32, name="g0n_ps", tag="pwe")
            mm512(lambda o, n: g0n_ps[0:64, o:o + n], g0_wihT[:, 128:192],
                  lambda o, n: x0r[:, o:o + n], NPC, True, True)
            gin0 = wp.tile([64, NPC], f16, name="gin0", tag="gru_gin")
            nc.scalar.activation(gin0[:], g0n_ps[0:64, :], AF.Identity, bias=g0_bihn)
            # h=0 so gh_n = bhh_n: broadcast bhh_n across columns (scale=0 trick)
            hn0 = wp.tile([64, NPC], f16, name="hn0", tag="gru_hn")
            nc.scalar.activation(hn0[:], gin0[:], AF.Identity, bias=g0_bhhn, scale=0.0)
            h_T = gru_elem(rz0, gin0, hn0, None, "p0")

            h_transposes(h_T, 0, "p0", want_sbuf=False)
            nc.gpsimd.collective_compute(
                "AllGather", OP.bypass, replica_groups=RG,
                ins=[hown[0].opt()], outs=[hfull[0].opt()],
            )

            # ---------------- conv depths ----------------
            h_nm = None
            for d in range(DEPTHS):
                aggT = pagg.tile([64, NPC], f32, name=f"aggT{d}", tag="agg")
                # root contribution first: start=True zeroes the accumulator
                for s in range(2):
                    nc.tensor.matmul(
                        aggT[0:64, s * 512:(s + 1) * 512],
                        rootw[d],
                        h_T[:, s * 512:(s + 1) * 512],
                        start=True, stop=False,
                    )

                # all gathers first (own-src ones lead: hown is ready
                # before the AllGather lands), so no GPSIMD multiply ever
                # blocks a queued gather or vice versa
                hsfs = {}
                for t in range(T):
                    hsf = hsfp.tile([P, DIM], f16, name=f"hsf{d}_{t}", tag="hsf")
                    src_dram = hown[d] if t < T_OWN else hfull[d]
                    nc.gpsimd.indirect_dma_start(
                        out=hsf[:], out_offset=None,
                        in_=src_dram[:, :],
                        in_offset=bass.IndirectOffsetOnAxis(ap=srcx[:, t:t + 1], axis=0),
                    )
                    hsfs[t] = hsf
                def emit_scatter(t):
                    for s in range(2):
                        nc.tensor.matmul(
                            aggT[0:64, s * 512:(s + 1) * 512],
                            msgs[t][:],
                            S[:, t * NPC + s * 512: t * NPC + (s + 1) * 512],
                            start=False, stop=(t == T - 1),
                        )

                msgs = {}
                for t in range(T):
                    hsf = hsfs[t]
                    hv = hsf[:, :].rearrange("p (g l) -> p g l", l=8)
                    wsb = wsbp.tile([P, 4096], f16, name=f"wsb{d}_{t}", tag="wsb")
                    for q in range(4):
                        wps = pwe.tile([P, NPC], f32, name=f"we{d}_{t}_{q}", tag="pwe")
                        mm512(lambda o, n, _q=q, _t=t: wps[:, o:o + n],
                              hidT[d][:, t * P:(t + 1) * P],
                              lambda o, n, _q=q: w2p[d][:, _q * 1024 + o:_q * 1024 + o + n],
                              1024, True, True)
                        nc.scalar.activation(
                            wsb[:, q * 1024:(q + 1) * 1024], wps[:], AF.Copy
                        )
                        if b2bc is not None:
                            nc.vector.tensor_tensor(
                                out=wsb[:, q * 1024:(q + 1) * 1024],
                                in0=wsb[:, q * 1024:(q + 1) * 1024],
                                in1=b2bc[d][:, q * 1024:(q + 1) * 1024], op=OP.add,
                            )
                    nc.vector.tensor_tensor(
                        out=wsb[:].rearrange("p (g o l) -> p g o l", o=64, l=8),
                        in0=wsb[:].rearrange("p (g o l) -> p g o l", o=64, l=8),
                        in1=hv[:, :, :].unsqueeze(2).to_broadcast([P, 8, 64, 8]),
                        op=OP.mult,
                    )
                    # fold-adds over the i_hi bits are contiguous-slice
                    # in-place adds (w2p column layout is (i_hi3, o, i_lo3))
                    for w in (2048, 1024, 512):
                        nc.vector.tensor_tensor(
                            out=wsb[:, 0:w], in0=wsb[:, 0:w], in1=wsb[:, w:2 * w],
                            op=OP.add,
                        )
                    msg = ep.tile([P, DIM], f16, name=f"msg{d}_{t}", tag="msg")
                    nc.vector.tensor_reduce(
                        out=msg[:], in_=wsb[:, 0:512].rearrange("p (o l) -> p o l", l=8),
                        axis=mybir.AxisListType.X, op=OP.add,
                    )
                    msgs[t] = msg
                    # scatter matmuls trail by SCAT_LAG tiles so one late msg
                    # never head-of-line-blocks the PE queue's We matmuls
                    if t >= SCAT_LAG:
                        emit_scatter(t - SCAT_LAG)
                for t in range(max(0, T - SCAT_LAG), T):
                    emit_scatter(t)
                xc = wp.tile([64, NPC], f16, name=f"xc{d}", tag="xc")
                nc.scalar.activation(xc[:], aggT[0:64, :], AF.Relu, bias=convb[d])

                # ---- GRU(xc, h): r/z gates accumulate wih@x + whh@h in PSUM
                rz_ps = pwe.tile([P, NPC], f32, name=f"rz{d}", tag="pwe")
                for s in range(2):
                    nc.tensor.matmul(
                        rz_ps[0:128, s * 512:(s + 1) * 512], wihT[d][:, 0:128],
                        xc[:, s * 512:(s + 1) * 512], start=True, stop=False,
                    )
                    nc.tensor.matmul(
                        rz_ps[0:128, s * 512:(s + 1) * 512], whhT[d][:, 0:128],
                        h_T[:, s * 512:(s + 1) * 512], start=False, stop=True,
                    )
                rz = wp.tile([P, NPC], f16, name=f"rzs{d}", tag="gru_rz")
                nc.scalar.activation(rz[:], rz_ps[0:128, :], AF.Sigmoid, bias=brz[d])

                gin_ps = pwe.tile([P, NPC], f32, name=f"gin{d}", tag="pwe")
                mm512(lambda o, n: gin_ps[0:64, o:o + n], wihT[d][:, 128:192],
                      lambda o, n: xc[:, o:o + n], NPC, True, True)
                gin = wp.tile([64, NPC], f16, name=f"gins{d}", tag="gru_gin")
                nc.scalar.activation(gin[:], gin_ps[0:64, :], AF.Identity, bias=bihn[d])

                ghn_ps = pwe.tile([P, NPC], f32, name=f"ghn{d}", tag="pwe")
                mm512(lambda o, n: ghn_ps[0:64, o:o + n], whhT[d][:, 128:192],
                      lambda o, n: h_T[:, o:o + n], NPC, True, True)
                hn = wp.tile([64, NPC], f16, name=f"hns{d}", tag="gru_hn")
                nc.scalar.activation(hn[:], ghn_ps[0:64, :], AF.Identity, bias=bhhn[d])
                h_T = gru_elem(rz, gin, hn, h_T, f"d{d}")

                if d < DEPTHS - 1:
                    h_transposes(h_T, d + 1, f"d{d}", want_sbuf=False)
                    nc.gpsimd.collective_compute(
                        "AllGather", OP.bypass, replica_groups=RG,
                        ins=[hown[d + 1].opt()], outs=[hfull[d + 1].opt()],
                    )
                else:
                    h_nm = h_transposes(h_T, None, f"d{d}", want_sbuf=True)
                    pooled_ps = pagg.tile([64, N_GRAPHS], f32, name="pooled_ps", tag="agg")
                    for c in range(8):
                        nc.tensor.matmul(
                            pooled_ps[0:64, :],
                            h_nm[c][:],
                            pS[:, c * N_GRAPHS:(c + 1) * N_GRAPHS],
                            start=(c == 0), stop=(c == 7),
                        )
                    pooled_sb = wp.tile([64, N_GRAPHS], f32, name="pooled_sb")
                    nc.scalar.activation(pooled_sb[:], pooled_ps[0:64, :], AF.Copy)
                    nc.sync.dma_start(ar_in[:, :], pooled_sb[:])

            # ---------------- pooling AllReduce + output MLP ----------------
            nc.gpsimd.collective_compute(
                "AllReduce", OP.add, replica_groups=RG,
                ins=[ar_in.opt()], outs=[ar_out.opt()],
            )
            pooled = wp.tile([64, N_GRAPHS], f32, name="pooled")
            nc.sync.dma_start(pooled[:], ar_out[:, :])
            pooled16 = wp.tile([64, N_GRAPHS], f16, name="pooled16")
            nc.scalar.activation(pooled16[:], pooled[:], AF.Copy)

            m1_ps = pagg.tile([64, N_GRAPHS], f32, name="m1_ps", tag="agg")
            nc.tensor.matmul(m1_ps[0:64, :], o0wT, pooled16[:], start=True, stop=True)
            m1r = wp.tile([64, N_GRAPHS], f16, name="m1r")
            nc.scalar.activation(m1r[:], m1_ps[0:64, :], AF.Relu, bias=o0b)

            m2_ps = pagg.tile([64, N_GRAPHS], f32, name="m2_ps", tag="agg")
            nc.tensor.matmul(m2_ps[0:32, :], o1wT, m1r[:], start=True, stop=True)
            m2b = wp.tile([32, N_GRAPHS], f16, name="m2b")
            nc.scalar.activation(m2b[:], m2_ps[0:32, :], AF.Identity, bias=o1b)

            m3_ps = pagg.tile([64, N_GRAPHS], f32, name="m3_ps", tag="agg")
            nc.tensor.matmul(m3_ps[0:1, :], o2wT, m2b[:], start=True, stop=True)
            ysb = wp.tile([1, N_GRAPHS], f32, name="ysb")
            nc.scalar.activation(ysb[:], m3_ps[0:1, :], AF.Identity, bias=o2b)
            nc.sync.dma_start(y_d[:, :], ysb[:])

    nc.finalize()
    return nc


def _prep(inputs):
    """Host-side sharding + weight permutation. Returns (T, T_OWN, b2_zero, in_maps)."""
    g = lambda k: np.asarray(inputs[k])
    x = g("x").astype(np.float32)
    ea = g("edge_attr").astype(np.float32)
    ei = g("edge_index").astype(np.int64)
    batch = g("batch").astype(np.int64)
    src, dst = ei[0], ei[1]

    owner = dst // NPC
    core_ids = [np.nonzero(owner == c)[0] for c in range(NC)]

    # own-src edges (src owned by the same core) are ordered first; they can
    # gather h from the core-local copy before the AllGather completes.
    own_lists, gen_lists = [], []
    for c in range(NC):
        ids = core_ids[c]
        is_own = (src[ids] // NPC) == c
        own_lists.append(ids[is_own])
        gen_lists.append(ids[~is_own])
    min_own = min(len(o) for o in own_lists)
    T_OWN = max(1, min(2, (min_own + P - 1) // P))
    n_own_slots = T_OWN * P

    seqs = []
    for c in range(NC):
        own, gen = own_lists[c], gen_lists[c]
        own_used = own[:n_own_slots]
        spill = own[n_own_slots:]
        gen_all = np.concatenate([spill, gen])
        seqs.append((own_used, gen_all))
    T_GEN = max((len(gl) + P - 1) // P for _, gl in seqs)
    T = T_OWN + T_GEN
    EP = T * P

    cnt = np.bincount(batch, minlength=N_GRAPHS).astype(np.float32)
    inv = 1.0 / np.maximum(cnt, 1.0)

    mlp2_b = g("mlp2_b").astype(np.float32)
    b2_zero = bool(np.all(mlp2_b == 0))

    # ---- shared weights: small tensors packed into two DMA payloads
    mlp1_w = g("mlp1_w").astype(np.float32)
    mlp1_b = g("mlp1_b").astype(np.float32)
    mlp2_w = g("mlp2_w").astype(np.float32)
    root_w = g("root_w").astype(np.float32)
    conv_b = g("conv_b").astype(np.float32)
    gru_wih = g("gru_wih").astype(np.float32)
    gru_whh = g("gru_whh").astype(np.float32)
    gru_bih = g("gru_bih").astype(np.float32)
    gru_bhh = g("gru_bhh").astype(np.float32)

    bias_pack = np.zeros((P, 22), np.float32)
    bj = [0]
    def bput(v):
        bias_pack[:len(v), bj[0]] = v; bj[0] += 1
    bput(g("fc0_b").astype(np.float32))
    bput((g("gru0_bih") + g("gru0_bhh")).astype(np.float32)[:128])
    bput(g("gru0_bih").astype(np.float32)[128:])
    bput(g("gru0_bhh").astype(np.float32)[128:])
    for d in range(DEPTHS): bput(mlp1_b[d])
    for d in range(DEPTHS): bput(conv_b[d])
    for d in range(DEPTHS): bput((gru_bih[d] + gru_bhh[d])[:128])
    for d in range(DEPTHS): bput(gru_bih[d][128:])
    for d in range(DEPTHS): bput(gru_bhh[d][128:])
    bput(g("out0_b").astype(np.float32))
    bput(g("out1_b").astype(np.float32))
    bput(g("out2_b").astype(np.float32))

    wpack = np.zeros((64, 2049), np.float16)
    wj = [0]
    def wput(m):
        r, c = m.shape
        wpack[:r, wj[0]:wj[0] + c] = m.astype(np.float16); wj[0] += c
    wput(g("fc0_w").astype(np.float32).T)
    wput(g("gru0_wih").astype(np.float32).T)
    for d in range(DEPTHS): wput(mlp1_w[d].T)
    for d in range(DEPTHS): wput(root_w[d])
    for d in range(DEPTHS): wput(gru_wih[d].T)
    for d in range(DEPTHS): wput(gru_whh[d].T)
    wput(g("out0_w").astype(np.float32).T)
    wput(g("out1_w").astype(np.float32).T)
    wput(g("out2_w").astype(np.float32).T)

    shared = {"biaspack": bias_pack, "wpack16": wpack}
    for d in range(DEPTHS):
        # column layout (i_hi3, o, i_lo3): fold-adds over i become
        # contiguous-slice adds (DVE 2x mode needs packed operands)
        shared[f"w2p{d}"] = (
            mlp2_w[d].reshape(8, 8, 64, 128).transpose(3, 0, 2, 1).reshape(128, 4096)
        ).astype(np.float16)
        if not b2_zero:
            b2p = mlp2_b[d].reshape(8, 8, 64).transpose(0, 2, 1).reshape(4096)
            shared[f"b2bc{d}"] = np.broadcast_to(
                b2p.astype(np.float16), (P, 4096)
            ).copy()

    in_maps = []
    for c in range(NC):
        own_used, gen_all = seqs[c]
        ids = np.concatenate([own_used, np.full(n_own_slots - len(own_used), -1),
                              gen_all, np.full(EP - n_own_slots - len(gen_all), -1)])
        valid = ids >= 0
        idv = ids.copy()
        idv[~valid] = 0  # placeholder edge (zero S column kills contribution)
        src_pad = src[idv].astype(np.int32)
        src_pad[~valid] = c * NPC  # any in-range node
        # own-src tiles use LOCAL indices into hown
        src_pad[:n_own_slots] -= c * NPC
        ea_pad = ea[idv].astype(np.float32)
        ea_pad[~valid] = 0.0
        S_full = np.zeros((EP, NPC), np.float16)
        rows = np.nonzero(valid)[0]
        S_full[rows, dst[idv[rows]] - c * NPC] = 1.0
        S_tab = np.zeros((P, T * NPC), np.float16)
        for t in range(T):
            S_tab[:, t * NPC:(t + 1) * NPC] = S_full[t * P:(t + 1) * P]
        pm_full = np.zeros((NPC, N_GRAPHS), np.float16)
        nb = batch[c * NPC:(c + 1) * NPC]
        pm_full[np.arange(NPC), nb] = inv[nb].astype(np.float16)
        # pre-chunked [(128), 8*64]: chunk cc covers nodes cc*128..+128
        pm = np.zeros((P, 8 * N_GRAPHS), np.float16)
        for cc in range(8):
            pm[:, cc * N_GRAPHS:(cc + 1) * N_GRAPHS] = pm_full[cc * P:(cc + 1) * P]
        m = {
            "xT": x[c * NPC:(c + 1) * NPC].T.astype(np.float16).copy(),
            "eaT": ea_pad.T.astype(np.float16).copy(),
            "srcidx": src_pad.reshape(T, P).T.copy(),
            "S": S_tab,
            "poolS": pm,
        }
        m.update(shared)
        in_maps.append(m)
    return T, T_OWN, b2_zero, in_maps


def kernel(**inputs) -> np.ndarray:
    global LAST_EXEC_NS, LAST_RESULTS
    T, T_OWN, b2_zero, in_maps = _prep(inputs)
    key = (T, T_OWN, b2_zero)
    if key not in _CACHE:
        _CACHE[key] = _build(T, T_OWN, b2_zero)
    nc = _CACHE[key]

    from concourse.bass_utils import run_bass_kernel_spmd

    if TRACE:
        res = run_bass_kernel_spmd(
            nc, in_maps, list(range(NC)), trace=True, trace_cores=list(range(NC))
        )
        LAST_EXEC_NS = res.exec_time_ns
        LAST_RESULTS = res
    else:
        res = run_bass_kernel_spmd(nc, in_maps, list(range(NC)))
    return res.results[0]["y"].reshape(N_GRAPHS).astype(np.float32)


# revision 17
# speedup vs baseline: 1.4698x; 1.0265x over previous
"""NNConv+GRU message-passing network (ConvGRU) on 8 Trainium2 NeuronCores.

Strategy (v2, tuned from trace analysis of the v1 baseline):
  - Edges sharded by OWNER OF DST node (8 node ranges of 1024); scatter-add
    realized as matmul against a 0/1 selection matrix (exact dup handling).
  - h node-sharded for the GRU; AllGathered (fp16) once per conv layer.
    Edges whose SRC is also core-local are ordered first and gather h from
    the local copy, giving DVE work during the AllGather latency.
  - Per-edge weights We: PE computes hid@w2p into PSUM fp32 (fp16 inputs),
    ACT evacuates to one fp16 [128,4096] SBUF tile per edge-tile, then the
    per-edge matvec is: one broadcast multiply (DVE 2x mode, or GPSIMD for
    a subset of tiles to balance engines), three in-place strided fold-adds
    (DVE 2x), and one short tensor_reduce -> fp16 msg.
  - Everything on the h path is fp16 (fp32 matmuls cost 4 cyc/col vs 1).
  - GRU r/z: the wih@x and whh@h matmuls accumulate into one PSUM tile.

Self-contained: only needs numpy + the concourse/bass stack installed in the
container. All shapes hardcoded for this problem size.
"""
import numpy as np

DIM = 64
DEPTHS = 3
N_NODES = 8192
N_EDGES = 16384
N_GRAPHS = 64
NC = 8
NPC = N_NODES // NC   # 1024 nodes per core
P = 128

TRACE = False
LAST_EXEC_NS = None
LAST_RESULTS = None

_CACHE = {}

# tiles t (past the own-src block) with t % 8 in this set do their broadcast
# multiply on GPSIMD instead of DVE (engine balancing)
GPS_PAT = ()
SCAT_LAG = 3


def _build(T, T_OWN, b2_zero):
    """Build the (shared) 8-core SPMD program. Per-core data arrives via inputs."""
    import concourse.mybir as mybir
    import concourse.tile as tile
    from concourse import bacc
    import concourse.bass as bass
    from concourse.masks import make_identity

    f32 = mybir.dt.float32
    f16 = mybir.dt.float16
    i32 = mybir.dt.int32
    AF = mybir.ActivationFunctionType
    OP = mybir.AluOpType
    EP = T * P  # padded edge count per core

    nc = bacc.Bacc("TRN2", target_bir_lowering=False, debug=False, num_devices=NC)

    def din(name, shape, dt=f32):
        return nc.dram_tensor(name, shape, dt, kind="ExternalInput")

    xT_d = din("xT", [40, NPC], f16)
    eaT_d = din("eaT", [10, EP], f16)
    srcx_d = din("srcidx", [P, T], i32)
    S_d = din("S", [P, T * NPC], f16)
    pS_d = din("poolS", [P, 8 * N_GRAPHS], f16)
    # all small fp32 bias vectors as columns of one tensor (one DMA);
    # all small fp16 weight mats packed along columns of one tensor
    bp_d = din("biaspack", [P, 22])
    wp16_d = din("wpack16", [64, 2049], f16)
    w2p_d = [din(f"w2p{d}", [128, 4096], f16) for d in range(DEPTHS)]
    b2bc_d = None if b2_zero else [din(f"b2bc{d}", [128, 4096], f16) for d in range(DEPTHS)]

    y_d = nc.dram_tensor("y", [1, N_GRAPHS], f32, kind="ExternalOutput")

    RG = [list(range(NC))]

    with nc.allow_low_precision("fp16 pipeline; final tolerance is 2e-2"), \
         tile.TileContext(nc) as tc:
        with (
            tc.tile_pool(name="const", bufs=1) as cp,
            tc.tile_pool(name="work", bufs=2) as wp,
            tc.tile_pool(name="wsbp", bufs=6) as wsbp,
            tc.tile_pool(name="edge", bufs=6) as ep,
            tc.tile_pool(name="hsfp", bufs=T + 3) as hsfp,
            tc.tile_pool(name="pwe", bufs=2, space="PSUM") as pwe,
            tc.tile_pool(name="pagg", bufs=1, space="PSUM") as pagg,
            tc.tile_pool(name="ptp", bufs=2, space="PSUM") as ptp,
            tc.tile_pool(name="dram", bufs=1, space="DRAM") as dp,
        ):
            # ---------------- constants to SBUF ----------------
            def load(name, dram, shape, dt=f32, ap=None):
                t = cp.tile(shape, dt, name=name)
                nc.sync.dma_start(t[:], dram[:, :] if ap is None else ap)
                return t

            # load order = DMA issue order (Sync issues serially at
            # ~0.6-1us each, so everything small is packed into two DMAs)
            xT = load("xT_s", xT_d, [40, NPC], f16)
            bpk = load("bp_s", bp_d, [P, 22])
            wpk = load("wp16_s", wp16_d, [64, 2049], f16)
            eaT = load("eaT_s", eaT_d, [10, EP], f16)
            srcx = load("srcx_s", srcx_d, [P, T], i32)
            w2p = [load(f"w2p_s{d}", w2p_d[d], [128, 4096], f16) for d in range(DEPTHS)]
            S = load("S_s", S_d, [P, T * NPC], f16)
            pS = load("pS_s", pS_d, [P, 8 * N_GRAPHS], f16)
            b2bc = (
                None if b2_zero else
                [load(f"b2bc_s{d}", b2bc_d[d], [128, 4096], f16) for d in range(DEPTHS)]
            )

            bcol = [0]
            def bslice(rows):
                j = bcol[0]; bcol[0] += 1
                return bpk[0:rows, j:j + 1]
            fc0_b = bslice(32)
            g0_brz = bslice(128)
            g0_bihn = bslice(64)
            g0_bhhn = bslice(64)
            m1b = [bslice(128) for d in range(DEPTHS)]
            convb = [bslice(64) for d in range(DEPTHS)]
            brz = [bslice(128) for d in range(DEPTHS)]
            bihn = [bslice(64) for d in range(DEPTHS)]
            bhhn = [bslice(64) for d in range(DEPTHS)]
            o0b = bslice(64)
            o1b = bslice(32)
            o2b = bslice(1)

            wcol = [0]
            def wslice(rows, cols):
                j = wcol[0]; wcol[0] += cols
                return wpk[0:rows, j:j + cols]
            fc0_wT = wslice(40, 32)
            g0_wihT = wslice(32, 192)
            m1wT = [wslice(10, 128) for d in range(DEPTHS)]
            rootw = [wslice(64, 64) for d in range(DEPTHS)]
            wihT = [wslice(64, 192) for d in range(DEPTHS)]
            whhT = [wslice(64, 192) for d in range(DEPTHS)]
            o0wT = wslice(64, 64)
            o1wT = wslice(64, 32)
            o2wT = wslice(32, 1)

            ident = cp.tile([64, 64], f16, name="ident")
            make_identity(nc, ident[:])

            # a zero-byte-ish collective right at program start absorbs the
            # one-time global barrier/rendezvous cost while cores are still
            # loading constants, instead of stalling the first AllGather
            warm_in = dp.tile([1, 8], f32, name="warm_in")
            warm_out = dp.tile([8, 8], f32, name="warm_out")
            wtile = wp.tile([1, 8], f32, name="wtile")
            nc.gpsimd.memset(wtile[:], 0.0)
            nc.sync.dma_start(warm_in[:, :], wtile[:])
            nc.gpsimd.collective_compute(
                "AllGather", OP.bypass, replica_groups=RG,
                ins=[warm_in.opt()], outs=[warm_out.opt()],
            )
            hown = [dp.tile([NPC, DIM], f16, name=f"hown{d}") for d in range(DEPTHS)]
            hfull = [dp.tile([N_NODES, DIM], f16, name=f"hfull{d}") for d in range(DEPTHS)]
            ar_in = dp.tile([DIM, N_GRAPHS], f32, name="ar_in")
            ar_out = dp.tile([DIM, N_GRAPHS], f32, name="ar_out")

            # ---------------- helpers ----------------
            def mm512(out_ap_fn, lhsT, rhs_fn, n_total, start, stop):
                """matmuls in 512-wide chunks: out[:, s] = lhsT.T @ rhs[:, s]."""
                off = 0
                while off < n_total:
                    n = min(512, n_total - off)
                    nc.tensor.matmul(
                        out_ap_fn(off, n), lhsT, rhs_fn(off, n),
                        start=start, stop=stop,
                    )
                    off += n

            def gru_elem(rz_s, gi_n_s, hn_s, h_prev, tagp):
                """rz_s [128,1024] f16 (r||z post-sigmoid), gi_n_s/hn_s [64,1024] f16.
                Returns new h_T [64,1024] f16: h' = n + z*(h - n)."""
                # DVE needs equal base partitions for SBUF+SBUF tensor_tensor,
                # so shift the z half down to a base-0 tile via SBUF->SBUF DMA.
                z_s = wp.tile([64, NPC], f16, name=f"z_{tagp}", tag="gru_z")
                nc.sync.dma_start(z_s[:], rz_s[64:128, :])
                t1 = wp.tile([64, NPC], f16, name=f"t1_{tagp}", tag="gru_t1")
                nc.vector.tensor_tensor(out=t1[:], in0=rz_s[0:64, :], in1=hn_s[:], op=OP.mult)
                nc.vector.tensor_tensor(out=t1[:], in0=t1[:], in1=gi_n_s[:], op=OP.add)
                nt = wp.tile([64, NPC], f16, name=f"nt_{tagp}", tag="gru_nt")
                nc.scalar.activation(nt[:], t1[:], AF.Tanh)
                hm = wp.tile([64, NPC], f16, name=f"hm_{tagp}", tag="gru_hm")
                if h_prev is None:
                    # h=0: h' = n - z*n
                    nc.vector.tensor_tensor(out=hm[:], in0=z_s[:], in1=nt[:], op=OP.mult)
                    hnew = wp.tile([64, NPC], f16, name=f"h_{tagp}", tag="hT")
                    nc.vector.tensor_tensor(out=hnew[:], in0=nt[:], in1=hm[:], op=OP.subtract)
                else:
                    nc.vector.tensor_tensor(out=hm[:], in0=h_prev[:], in1=nt[:], op=OP.subtract)
                    nc.vector.tensor_tensor(out=hm[:], in0=hm[:], in1=z_s[:], op=OP.mult)
                    hnew = wp.tile([64, NPC], f16, name=f"h_{tagp}", tag="hT")
                    nc.vector.tensor_tensor(out=hnew[:], in0=hm[:], in1=nt[:], op=OP.add)
                return hnew

            def h_transposes(h_T, d_out, tagp, want_sbuf):
                """PE-transpose h_T [64,1024] f16 -> 8 [128,64] node-major SBUF
                tiles (PSUM can't feed DMA directly), DMA each to hown[d_out]."""
                sb = []
                for c in range(8):
                    tp = ptp.tile([P, DIM], f16, name=f"tp_{tagp}_{c}", tag="tp")
                    nc.tensor.transpose(
                        out=tp[:], in_=h_T[:, c * P:(c + 1) * P], identity=ident[:]
                    )
                    hm = wp.tile([P, DIM], f16, name=f"hnm_{tagp}_{c}", tag=f"hnm{c}")
                    if c % 2 == 0:
                        nc.scalar.activation(hm[:], tp[:], AF.Copy)
                    else:
                        nc.vector.tensor_copy(hm[:], tp[:])
                    if d_out is not None:
                        nc.sync.dma_start(hown[d_out][c * P:(c + 1) * P, :], hm[:])
                    if want_sbuf:
                        sb.append(hm)
                return sb

            # ---------------- edge-MLP hidden states, all depths upfront ----
            hidT = []
            for d in range(DEPTHS):
                ht = cp.tile([P, EP], f16, name=f"hidT{d}")
                off = 0
                while off < EP:
                    n = min(1024, EP - off)
                    hp = pwe.tile([P, NPC], f32, name=f"hid_ps{d}_{off}", tag="pwe")
                    mm512(lambda o, nn, _b=off: hp[:, o:o + nn], m1wT[d],
                          lambda o, nn, _b=off: eaT[:, _b + o:_b + o + nn], n, True, True)
                    nc.scalar.activation(
                        ht[:, off:off + n], hp[:, 0:n], AF.Relu, bias=m1b[d]
                    )
                    off += n
                hidT.append(ht)

            # ---------------- phase 0: fc0 + gru0 (h0 = 0) ----------------
            x0_ps = pwe.tile([P, NPC], f32, name="x0_ps", tag="pwe")
            mm512(lambda o, n: x0_ps[0:32, o:o + n], fc0_wT,
                  lambda o, n: xT[:, o:o + n], NPC, True, True)
            x0r = wp.tile([32, NPC], f16, name="x0r")
            nc.scalar.activation(x0r[:], x0_ps[0:32, :], AF.Relu, bias=fc0_b)

            g0rz_ps = pwe.tile([P, NPC], f32, name="g0rz_ps", tag="pwe")
            mm512(lambda o, n: g0rz_ps[0:128, o:o + n], g0_wihT[:, 0:128],
                  lambda o, n: x0r[:, o:o + n], NPC, True, True)
            rz0 = wp.tile([P, NPC], f16, name="rz0", tag="gru_rz")
            nc.scalar.activation(rz0[:], g0rz_ps[0:128, :], AF.Sigmoid, bias=g0_brz)

            g0n_ps = pwe.tile([P, NPC], f32, name="g0n_ps", tag="pwe")
            mm512(lambda o, n: g0n_ps[0:64, o:o + n], g0_wihT[:, 128:192],
                  lambda o, n: x0r[:, o:o + n], NPC, True, True)
            gin0 = wp.tile([64, NPC], f16, name="gin0", tag="gru_gin")
            nc.scalar.activation(gin0[:], g0n_ps[0:64, :], AF.Identity, bias=g0_bihn)
            # h=0 so gh_n = bhh_n: broadcast bhh_n across columns (scale=0 trick)
            hn0 = wp.tile([64, NPC], f16, name="hn0", tag="gru_hn")
            nc.scalar.activation(hn0[:], gin0[:], AF.Identity, bias=g0_bhhn, scale=0.0)
            h_T = gru_elem(rz0, gin0, hn0, None, "p0")

            h_transposes(h_T, 0, "p0", want_sbuf=False)
            nc.gpsimd.collective_compute(
                "AllGather", OP.bypass, replica_groups=RG,
                ins=[hown[0].opt()], outs=[hfull[0].opt()],
            )

            # ---------------- conv depths ----------------
            h_nm = None
            for d in range(DEPTHS):
                aggT = pagg.tile([64, NPC], f32, name=f"aggT{d}", tag="agg")
                # root contribution first: start=True zeroes the accumulator
                for s in range(2):
                    nc.tensor.matmul(
                        aggT[0:64, s * 512:(s + 1) * 512],
                        rootw[d],
                        h_T[:, s * 512:(s + 1) * 512],
                        start=True, stop=False,
                    )

                # all gathers first (own-src ones lead: hown is ready
                # before the AllGather lands), so no GPSIMD multiply ever
                # blocks a queued gather or vice versa
                hsfs = {}
                for t in range(T):
                    hsf = hsfp.tile([P, DIM], f16, name=f"hsf{d}_{t}", tag="hsf")
                    src_dram = hown[d] if t < T_OWN else hfull[d]
                    nc.gpsimd.indirect_dma_start(
                        out=hsf[:], out_offset=None,
                        in_=src_dram[:, :],
                        in_offset=bass.IndirectOffsetOnAxis(ap=srcx[:, t:t + 1], axis=0),
                    )
                    hsfs[t] = hsf
                def emit_scatter(t):
                    for s in range(2):
                        nc.tensor.matmul(
                            aggT[0:64, s * 512:(s + 1) * 512],
                            msgs[t][:],
                            S[:, t * NPC + s * 512: t * NPC + (s + 1) * 512],
                            start=False, stop=(t == T - 1),
                        )

                msgs = {}
                for t in range(T):
                    hsf = hsfs[t]
                    hv = hsf[:, :].rearrange("p (g l) -> p g l", l=8)
                    wsb = wsbp.tile([P, 4096], f16, name=f"wsb{d}_{t}", tag="wsb")
                    for q in range(4):
                        wps = pwe.tile([P, NPC], f32, name=f"we{d}_{t}_{q}", tag="pwe")
                        mm512(lambda o, n, _q=q, _t=t: wps[:, o:o + n],
                              hidT[d][:, t * P:(t + 1) * P],
                              lambda o, n, _q=q: w2p[d][:, _q * 1024 + o:_q * 1024 + o + n],
                              1024, True, True)
                        nc.scalar.activation(
                            wsb[:, q * 1024:(q + 1) * 1024], wps[:], AF.Copy
                        )
                        if b2bc is not None:
                            nc.vector.tensor_tensor(
                                out=wsb[:, q * 1024:(q + 1) * 1024],
                                in0=wsb[:, q * 1024:(q + 1) * 1024],
                                in1=b2bc[d][:, q * 1024:(q + 1) * 1024], op=OP.add,
                            )
                    nc.vector.tensor_tensor(
                        out=wsb[:].rearrange("p (g o l) -> p g o l", o=64, l=8),
                        in0=wsb[:].rearrange("p (g o l) -> p g o l", o=64, l=8),
                        in1=hv[:, :, :].unsqueeze(2).to_broadcast([P, 8, 64, 8]),
                        op=OP.mult,
                    )
                    # fold-adds over the i_hi bits are contiguous-slice
                    # in-place adds (w2p column layout is (i_hi3, o, i_lo3))
                    for w in (2048, 1024, 512):
                        nc.vector.tensor_tensor(
                            out=wsb[:, 0:w], in0=wsb[:, 0:w], in1=wsb[:, w:2 * w],
                            op=OP.add,
                        )
                    msg = ep.tile([P, DIM], f16, name=f"msg{d}_{t}", tag="msg")
                    nc.vector.tensor_reduce(
                        out=msg[:], in_=wsb[:, 0:512].rearrange("p (o l) -> p o l", l=8),
                        axis=mybir.AxisListType.X, op=OP.add,
                    )
                    msgs[t] = msg
                    # scatter matmuls trail by SCAT_LAG tiles so one late msg
                    # never head-of-line-blocks the PE queue's We matmuls
                    if t >= SCAT_LAG:
                        emit_scatter(t - SCAT_LAG)
                for t in range(max(0, T - SCAT_LAG), T):
                    emit_scatter(t)
                xc = wp.tile([64, NPC], f16, name=f"xc{d}", tag="xc")
                nc.scalar.activation(xc[:], aggT[0:64, :], AF.Relu, bias=convb[d])

                # ---- GRU(xc, h): r/z gates accumulate wih@x + whh@h in PSUM
                rz_ps = pwe.tile([P, NPC], f32, name=f"rz{d}", tag="pwe")
                for s in range(2):
                    nc.tensor.matmul(
                        rz_ps[0:128, s * 512:(s + 1) * 512], wihT[d][:, 0:128],
                        xc[:, s * 512:(s + 1) * 512], start=True, stop=False,
                    )
                    nc.tensor.matmul(
                        rz_ps[0:128, s * 512:(s + 1) * 512], whhT[d][:, 0:128],
                        h_T[:, s * 512:(s + 1) * 512], start=False, stop=True,
                    )
                rz = wp.tile([P, NPC], f16, name=f"rzs{d}", tag="gru_rz")
                nc.scalar.activation(rz[:], rz_ps[0:128, :], AF.Sigmoid, bias=brz[d])

                gin_ps = pwe.tile([P, NPC], f32, name=f"gin{d}", tag="pwe")
                mm512(lambda o, n: gin_ps[0:64, o:o + n], wihT[d][:, 128:192],
                      lambda o, n: xc[:, o:o + n], NPC, True, True)
                gin = wp.tile([64, NPC], f16, name=f"gins{d}", tag="gru_gin")
                nc.scalar.activation(gin[:], gin_ps[0:64, :], AF.Identity, bias=bihn[d])

                ghn_ps = pwe.tile([P, NPC], f32, name=f"ghn{d}", tag="pwe")
                mm512(lambda o, n: ghn_ps[0:64, o:o + n], whhT[d][:, 128:192],
                      lambda o, n: h_T[:, o:o + n], NPC, True, True)
                hn = wp.tile([64, NPC], f16, name=f"hns{d}", tag="gru_hn")
                nc.scalar.activation(hn[:], ghn_ps[0:64, :], AF.Identity, bias=bhhn[d])
                h_T = gru_elem(rz, gin, hn, h_T, f"d{d}")

                if d < DEPTHS - 1:
                    h_transposes(h_T, d + 1, f"d{d}", want_sbuf=False)
                    nc.gpsimd.collective_compute(
                        "AllGather", OP.bypass, replica_groups=RG,
                        ins=[hown[d + 1].opt()], outs=[hfull[d + 1].opt()],
                    )
                else:
                    h_nm = h_transposes(h_T, None, f"d{d}", want_sbuf=True)
                    pooled_ps = pagg.tile([64, N_GRAPHS], f32, name="pooled_ps", tag="agg")
                    for c in range(8):
                        nc.tensor.matmul(
                            pooled_ps[0:64, :],
                            h_nm[c][:],
                            pS[:, c * N_GRAPHS:(c + 1) * N_GRAPHS],
                            start=(c == 0), stop=(c == 7),
                        )
                    pooled_sb = wp.tile([64, N_GRAPHS], f32, name="pooled_sb")
                    nc.scalar.activation(pooled_sb[:], pooled_ps[0:64, :], AF.Copy)
                    nc.sync.dma_start(ar_in[:, :], pooled_sb[:])

            # ---------------- pooling AllReduce + output MLP ----------------
            nc.gpsimd.collective_compute(
                "AllReduce", OP.add, replica_groups=RG,
                ins=[ar_in.opt()], outs=[ar_out.opt()],
            )
            pooled = wp.tile([64, N_GRAPHS], f32, name="pooled")
            nc.sync.dma_start(pooled[:], ar_out[:, :])
            pooled16 = wp.tile([64, N_GRAPHS], f16, name="pooled16")
            nc.scalar.activation(pooled16[:], pooled[:], AF.Copy)

            m1_ps = pagg.tile([64, N_GRAPHS], f32, name="m1_ps", tag="agg")
            nc.tensor.matmul(m1_ps[0:64, :], o0wT, pooled16[:], start=True, stop=True)
            m1r = wp.tile([64, N_GRAPHS], f16, name="m1r")
            nc.scalar.activation(m1r[:], m1_ps[0:64, :], AF.Relu, bias=o0b)

            m2_ps = pagg.tile([64, N_GRAPHS], f32, name="m2_ps", tag="agg")
            nc.tensor.matmul(m2_ps[0:32, :], o1wT, m1r[:], start=True, stop=True)
            m2b = wp.tile([32, N_GRAPHS], f16, name="m2b")
            nc.scalar.activation(m2b[:], m2_ps[0:32, :], AF.Identity, bias=o1b)

            m3_ps = pagg.tile([64, N_GRAPHS], f32, name="m3_ps", tag="agg")
            nc.tensor.matmul(m3_ps[0:1, :], o2wT, m2b[:], start=True, stop=True)
            ysb = wp.tile([1, N_GRAPHS], f32, name="ysb")
            nc.scalar.activation(ysb[:], m3_ps[0:1, :], AF.Identity, bias=o2b)
            nc.sync.dma_start(y_d[:, :], ysb[:])

    nc.finalize()
    return nc


def _prep(inputs):
    """Host-side sharding + weight permutation. Returns (T, T_OWN, b2_zero, in_maps)."""
    g = lambda k: np.asarray(inputs[k])
    x = g("x").astype(np.float32)
    ea = g("edge_attr").astype(np.float32)
    ei = g("edge_index").astype(np.int64)
    batch = g("batch").astype(np.int64)
    src, dst = ei[0], ei[1]

    owner = dst // NPC
    core_ids = [np.nonzero(owner == c)[0] for c in range(NC)]

    # own-src edges (src owned by the same core) are ordered first; they can
    # gather h from the core-local copy before the AllGather completes.
    own_lists, gen_lists = [], []
    for c in range(NC):
        ids = core_ids[c]
        is_own = (src[ids] // NPC) == c
        own_lists.append(ids[is_own])
        gen_lists.append(ids[~is_own])
    min_own = min(len(o) for o in own_lists)
    T_OWN = max(1, min(2, (min_own + P - 1) // P))
    n_own_slots = T_OWN * P

    seqs = []
    for c in range(NC):
        own, gen = own_lists[c], gen_lists[c]
        own_used = own[:n_own_slots]
        spill = own[n_own_slots:]
        gen_all = np.concatenate([spill, gen])
        seqs.append((own_used, gen_all))
    T_GEN = max((len(gl) + P - 1) // P for _, gl in seqs)
    T = T_OWN + T_GEN
    EP = T * P

    cnt = np.bincount(batch, minlength=N_GRAPHS).astype(np.float32)
    inv = 1.0 / np.maximum(cnt, 1.0)

    mlp2_b = g("mlp2_b").astype(np.float32)
    b2_zero = bool(np.all(mlp2_b == 0))

    # ---- shared weights: small tensors packed into two DMA payloads
    mlp1_w = g("mlp1_w").astype(np.float32)
    mlp1_b = g("mlp1_b").astype(np.float32)
    mlp2_w = g("mlp2_w").astype(np.float32)
    root_w = g("root_w").astype(np.float32)
    conv_b = g("conv_b").astype(np.float32)
    gru_wih = g("gru_wih").astype(np.float32)
    gru_whh = g("gru_whh").astype(np.float32)
    gru_bih = g("gru_bih").astype(np.float32)
    gru_bhh = g("gru_bhh").astype(np.float32)

    bias_pack = np.zeros((P, 22), np.float32)
    bj = [0]
    def bput(v):
        bias_pack[:len(v), bj[0]] = v; bj[0] += 1
    bput(g("fc0_b").astype(np.float32))
    bput((g("gru0_bih") + g("gru0_bhh")).astype(np.float32)[:128])
    bput(g("gru0_bih").astype(np.float32)[128:])
    bput(g("gru0_bhh").astype(np.float32)[128:])
    for d in range(DEPTHS): bput(mlp1_b[d])
    for d in range(DEPTHS): bput(conv_b[d])
    for d in range(DEPTHS): bput((gru_bih[d] + gru_bhh[d])[:128])
    for d in range(DEPTHS): bput(gru_bih[d][128:])
    for d in range(DEPTHS): bput(gru_bhh[d][128:])
    bput(g("out0_b").astype(np.float32))
    bput(g("out1_b").astype(np.float32))
    bput(g("out2_b").astype(np.float32))

    wpack = np.zeros((64, 2049), np.float16)
    wj = [0]
    def wput(m):
        r, c = m.shape
        wpack[:r, wj[0]:wj[0] + c] = m.astype(np.float16); wj[0] += c
    wput(g("fc0_w").astype(np.float32).T)
    wput(g("gru0_wih").astype(np.float32).T)
    for d in range(DEPTHS): wput(mlp1_w[d].T)
    for d in range(DEPTHS): wput(root_w[d])
    for d in range(DEPTHS): wput(gru_wih[d].T)
    for d in range(DEPTHS): wput(gru_whh[d].T)
    wput(g("out0_w").astype(np.float32).T)
    wput(g("out1_w").astype(np.float32).T)
    wput(g("out2_w").astype(np.float32).T)

    shared = {"biaspack": bias_pack, "wpack16": wpack}
    for d in range(DEPTHS):
        # column layout (i_hi3, o, i_lo3): fold-adds over i become
        # contiguous-slice adds (DVE 2x mode needs packed operands)
        shared[f"w2p{d}"] = (
            mlp2_w[d].reshape(8, 8, 64, 128).transpose(3, 0, 2, 1).reshape(128, 4096)
        ).astype(np.float16)
        if not b2_zero:
            b2p = mlp2_b[d].reshape(8, 8, 64).transpose(0, 2, 1).reshape(4096)
            shared[f"b2bc{d}"] = np.broadcast_to(
                b2p.astype(np.float16), (P, 4096)
            ).copy()

    in_maps = []
    for c in range(NC):
        own_used, gen_all = seqs[c]
        ids = np.concatenate([own_used, np.full(n_own_slots - len(own_used), -1),
                              gen_all, np.full(EP - n_own_slots - len(gen_all), -1)])
        valid = ids >= 0
        idv = ids.copy()
        idv[~valid] = 0  # placeholder edge (zero S column kills contribution)
        src_pad = src[idv].astype(np.int32)
        src_pad[~valid] = c * NPC  # any in-range node
        # own-src tiles use LOCAL indices into hown
        src_pad[:n_own_slots] -= c * NPC
        ea_pad = ea[idv].astype(np.float32)
        ea_pad[~valid] = 0.0
        S_full = np.zeros((EP, NPC), np.float16)
        rows = np.nonzero(valid)[0]
        S_full[rows, dst[idv[rows]] - c * NPC] = 1.0
        S_tab = np.zeros((P, T * NPC), np.float16)
        for t in range(T):
            S_tab[:, t * NPC:(t + 1) * NPC] = S_full[t * P:(t + 1) * P]
        pm_full = np.zeros((NPC, N_GRAPHS), np.float16)
        nb = batch[c * NPC:(c + 1) * NPC]
        pm_full[np.arange(NPC), nb] = inv[nb].astype(np.float16)
        # pre-chunked [(128), 8*64]: chunk cc covers nodes cc*128..+128
        pm = np.zeros((P, 8 * N_GRAPHS), np.float16)
        for cc in range(8):
            pm[:, cc * N_GRAPHS:(cc + 1) * N_GRAPHS] = pm_full[cc * P:(cc + 1) * P]
        m = {
            "xT": x[c * NPC:(c + 1) * NPC].T.astype(np.float16).copy(),
            "eaT": ea_pad.T.astype(np.float16).copy(),
            "srcidx": src_pad.reshape(T, P).T.copy(),
            "S": S_tab,
            "poolS": pm,
        }
        m.update(shared)
        in_maps.append(m)
    return T, T_OWN, b2_zero, in_maps


def kernel(**inputs) -> np.ndarray:
    global LAST_EXEC_NS, LAST_RESULTS
    T, T_OWN, b2_zero, in_maps = _prep(inputs)
    key = (T, T_OWN, b2_zero)
    if key not in _CACHE:
        _CACHE[key] = _build(T, T_OWN, b2_zero)
    nc = _CACHE[key]

    from concourse.bass_utils import run_bass_kernel_spmd

    if TRACE:
        res = run_bass_kernel_spmd(
            nc, in_maps, list(range(NC)), trace=True, trace_cores=list(range(NC))
        )
        LAST_EXEC_NS = res.exec_time_ns
        LAST_RESULTS = res
    else:
        res = run_bass_kernel_spmd(nc, in_maps, list(range(NC)))
    return res.results[0]["y"].reshape(N_GRAPHS).astype(np.float32)


# revision 19
# speedup vs baseline: 1.5020x; 1.0219x over previous
"""NNConv+GRU message-passing network (ConvGRU) on 8 Trainium2 NeuronCores.

Strategy (v2, tuned from trace analysis of the v1 baseline):
  - Edges sharded by OWNER OF DST node (8 node ranges of 1024); scatter-add
    realized as matmul against a 0/1 selection matrix (exact dup handling).
  - h node-sharded for the GRU; AllGathered (fp16) once per conv layer.
    Edges whose SRC is also core-local are ordered first and gather h from
    the local copy, giving DVE work during the AllGather latency.
  - Per-edge weights We: PE computes hid@w2p into PSUM fp32 (fp16 inputs),
    ACT evacuates to one fp16 [128,4096] SBUF tile per edge-tile, then the
    per-edge matvec is: one broadcast multiply (DVE 2x mode, or GPSIMD for
    a subset of tiles to balance engines), three in-place strided fold-adds
    (DVE 2x), and one short tensor_reduce -> fp16 msg.
  - Everything on the h path is fp16 (fp32 matmuls cost 4 cyc/col vs 1).
  - GRU r/z: the wih@x and whh@h matmuls accumulate into one PSUM tile.

Self-contained: only needs numpy + the concourse/bass stack installed in the
container. All shapes hardcoded for this problem size.
"""
import numpy as np

DIM = 64
DEPTHS = 3
N_NODES = 8192
N_EDGES = 16384
N_GRAPHS = 64
NC = 8
NPC = N_NODES // NC   # 1024 nodes per core
P = 128

TRACE = False
LAST_EXEC_NS = None
LAST_RESULTS = None

_CACHE = {}

# tiles t (past the own-src block) with t % 8 in this set do their broadcast
# multiply on GPSIMD instead of DVE (engine balancing)
GPS_PAT = ()
SCAT_LAG = 3


def _build(T, T_OWN, b2_zero):
    """Build the (shared) 8-core SPMD program. Per-core data arrives via inputs."""
    import concourse.mybir as mybir
    import concourse.tile as tile
    from concourse import bacc
    import concourse.bass as bass
    from concourse.masks import make_identity

    f32 = mybir.dt.float32
    f16 = mybir.dt.float16
    i32 = mybir.dt.int32
    AF = mybir.ActivationFunctionType
    OP = mybir.AluOpType
    EP = T * P  # padded edge count per core

    nc = bacc.Bacc("TRN2", target_bir_lowering=False, debug=False, num_devices=NC)

    def din(name, shape, dt=f32):
        return nc.dram_tensor(name, shape, dt, kind="ExternalInput")

    xT_d = din("xT", [40, NPC], f16)
    eaT_d = din("eaT", [10, EP], f16)
    srcx_d = din("srcidx", [P, T], i32)
    S_d = din("S", [P, T * NPC], f16)
    pS_d = din("poolS", [P, 8 * N_GRAPHS], f16)
    # all small fp32 bias vectors as columns of one tensor (one DMA);
    # all small fp16 weight mats packed along columns of one tensor
    bp_d = din("biaspack", [P, 22])
    wp16_d = din("wpack16", [64, 2049], f16)
    w2p_d = [din(f"w2p{d}", [128, 4096], f16) for d in range(DEPTHS)]
    b2bc_d = None if b2_zero else [din(f"b2bc{d}", [128, 4096], f16) for d in range(DEPTHS)]

    y_d = nc.dram_tensor("y", [1, N_GRAPHS], f32, kind="ExternalOutput")

    RG = [list(range(NC))]

    with nc.allow_low_precision("fp16 pipeline; final tolerance is 2e-2"), \
         tile.TileContext(nc) as tc:
        with (
            tc.tile_pool(name="const", bufs=1) as cp,
            tc.tile_pool(name="work", bufs=2) as wp,
            tc.tile_pool(name="wsbp", bufs=6) as wsbp,
            tc.tile_pool(name="edge", bufs=6) as ep,
            tc.tile_pool(name="hsfp", bufs=T + 3) as hsfp,
            tc.tile_pool(name="pwe", bufs=2, space="PSUM") as pwe,
            tc.tile_pool(name="pagg", bufs=1, space="PSUM") as pagg,
            tc.tile_pool(name="ptp", bufs=2, space="PSUM") as ptp,
            tc.tile_pool(name="dram", bufs=1, space="DRAM") as dp,
        ):
            # ---------------- constants to SBUF ----------------
            def load(name, dram, shape, dt=f32, ap=None):
                t = cp.tile(shape, dt, name=name)
                nc.sync.dma_start(t[:], dram[:, :] if ap is None else ap)
                return t

            # load order = DMA issue order (Sync issues serially at
            # ~0.6-1us each, so everything small is packed into two DMAs)
            xT = load("xT_s", xT_d, [40, NPC], f16)
            bpk = load("bp_s", bp_d, [P, 22])
            wpk = load("wp16_s", wp16_d, [64, 2049], f16)
            eaT = load("eaT_s", eaT_d, [10, EP], f16)
            srcx = load("srcx_s", srcx_d, [P, T], i32)
            w2p = [load(f"w2p_s{d}", w2p_d[d], [128, 4096], f16) for d in range(DEPTHS)]
            S = load("S_s", S_d, [P, T * NPC], f16)
            pS = load("pS_s", pS_d, [P, 8 * N_GRAPHS], f16)
            b2bc = (
                None if b2_zero else
                [load(f"b2bc_s{d}", b2bc_d[d], [128, 4096], f16) for d in range(DEPTHS)]
            )

            bcol = [0]
            def bslice(rows):
                j = bcol[0]; bcol[0] += 1
                return bpk[0:rows, j:j + 1]
            fc0_b = bslice(32)
            g0_brz = bslice(128)
            g0_bihn = bslice(64)
            g0_bhhn = bslice(64)
            m1b = [bslice(128) for d in range(DEPTHS)]
            convb = [bslice(64) for d in range(DEPTHS)]
            brz = [bslice(128) for d in range(DEPTHS)]
            bihn = [bslice(64) for d in range(DEPTHS)]
            bhhn = [bslice(64) for d in range(DEPTHS)]
            o0b = bslice(64)
            o1b = bslice(32)
            o2b = bslice(1)

            wcol = [0]
            def wslice(rows, cols):
                j = wcol[0]; wcol[0] += cols
                return wpk[0:rows, j:j + cols]
            fc0_wT = wslice(40, 32)
            g0_wihT = wslice(32, 192)
            m1wT = [wslice(10, 128) for d in range(DEPTHS)]
            rootw = [wslice(64, 64) for d in range(DEPTHS)]
            wihT = [wslice(64, 192) for d in range(DEPTHS)]
            whhT = [wslice(64, 192) for d in range(DEPTHS)]
            o0wT = wslice(64, 64)
            o1wT = wslice(64, 32)
            o2wT = wslice(32, 1)

            ident = cp.tile([64, 64], f16, name="ident")
            make_identity(nc, ident[:])

            # a zero-byte-ish collective right at program start absorbs the
            # one-time global barrier/rendezvous cost while cores are still
            # loading constants, instead of stalling the first AllGather
            warm_in = dp.tile([1, 8], f32, name="warm_in")
            warm_out = dp.tile([8, 8], f32, name="warm_out")
            wtile = wp.tile([1, 8], f32, name="wtile")
            nc.gpsimd.memset(wtile[:], 0.0)
            nc.sync.dma_start(warm_in[:, :], wtile[:])
            nc.gpsimd.collective_compute(
                "AllGather", OP.bypass, replica_groups=RG,
                ins=[warm_in.opt()], outs=[warm_out.opt()],
            )
            hown = [dp.tile([NPC, DIM], f16, name=f"hown{d}") for d in range(DEPTHS)]
            hfull = [dp.tile([N_NODES, DIM], f16, name=f"hfull{d}") for d in range(DEPTHS)]
            ar_in = dp.tile([DIM, N_GRAPHS], f32, name="ar_in")
            ar_out = dp.tile([DIM, N_GRAPHS], f32, name="ar_out")

            # ---------------- helpers ----------------
            def mm512(out_ap_fn, lhsT, rhs_fn, n_total, start, stop):
                """matmuls in 512-wide chunks: out[:, s] = lhsT.T @ rhs[:, s]."""
                off = 0
                while off < n_total:
                    n = min(512, n_total - off)
                    nc.tensor.matmul(
                        out_ap_fn(off, n), lhsT, rhs_fn(off, n),
                        start=start, stop=stop,
                    )
                    off += n

            def gru_elem(rz_s, gi_n_s, hn_s, h_prev, tagp):
                """rz_s [128,1024] f16 (r||z post-sigmoid), gi_n_s/hn_s [64,1024] f16.
                Returns new h_T [64,1024] f16: h' = n + z*(h - n)."""
                # DVE needs equal base partitions for SBUF+SBUF tensor_tensor,
                # so shift the z half down to a base-0 tile via SBUF->SBUF DMA.
                z_s = wp.tile([64, NPC], f16, name=f"z_{tagp}", tag="gru_z")
                nc.sync.dma_start(z_s[:], rz_s[64:128, :])
                t1 = wp.tile([64, NPC], f16, name=f"t1_{tagp}", tag="gru_t1")
                nc.vector.tensor_tensor(out=t1[:], in0=rz_s[0:64, :], in1=hn_s[:], op=OP.mult)
                nc.vector.tensor_tensor(out=t1[:], in0=t1[:], in1=gi_n_s[:], op=OP.add)
                nt = wp.tile([64, NPC], f16, name=f"nt_{tagp}", tag="gru_nt")
                nc.scalar.activation(nt[:], t1[:], AF.Tanh)
                hm = wp.tile([64, NPC], f16, name=f"hm_{tagp}", tag="gru_hm")
                if h_prev is None:
                    # h=0: h' = n - z*n
                    nc.vector.tensor_tensor(out=hm[:], in0=z_s[:], in1=nt[:], op=OP.mult)
                    hnew = wp.tile([64, NPC], f16, name=f"h_{tagp}", tag="hT")
                    nc.vector.tensor_tensor(out=hnew[:], in0=nt[:], in1=hm[:], op=OP.subtract)
                else:
                    nc.vector.tensor_tensor(out=hm[:], in0=h_prev[:], in1=nt[:], op=OP.subtract)
                    nc.vector.tensor_tensor(out=hm[:], in0=hm[:], in1=z_s[:], op=OP.mult)
                    hnew = wp.tile([64, NPC], f16, name=f"h_{tagp}", tag="hT")
                    nc.vector.tensor_tensor(out=hnew[:], in0=hm[:], in1=nt[:], op=OP.add)
                return hnew

            def h_transposes(h_T, d_out, tagp, want_sbuf):
                """PE-transpose h_T [64,1024] f16 -> 8 [128,64] node-major SBUF
                tiles (PSUM can't feed DMA directly), DMA each to hown[d_out]."""
                sb = []
                for c in range(8):
                    tp = ptp.tile([P, DIM], f16, name=f"tp_{tagp}_{c}", tag="tp")
                    nc.tensor.transpose(
                        out=tp[:], in_=h_T[:, c * P:(c + 1) * P], identity=ident[:]
                    )
                    hm = wp.tile([P, DIM], f16, name=f"hnm_{tagp}_{c}", tag=f"hnm{c}")
                    if c % 2 == 0:
                        nc.scalar.activation(hm[:], tp[:], AF.Copy)
                    else:
                        nc.vector.tensor_copy(hm[:], tp[:])
                    if d_out is not None:
                        nc.sync.dma_start(hown[d_out][c * P:(c + 1) * P, :], hm[:])
                    if want_sbuf:
                        sb.append(hm)
                return sb

            # ---------------- edge-MLP hidden states, all depths upfront ----
            hidT = []
            for d in range(DEPTHS):
                ht = cp.tile([P, EP], f16, name=f"hidT{d}")
                off = 0
                while off < EP:
                    n = min(1024, EP - off)
                    hp = pwe.tile([P, NPC], f32, name=f"hid_ps{d}_{off}", tag="pwe")
                    mm512(lambda o, nn, _b=off: hp[:, o:o + nn], m1wT[d],
                          lambda o, nn, _b=off: eaT[:, _b + o:_b + o + nn], n, True, True)
                    nc.scalar.activation(
                        ht[:, off:off + n], hp[:, 0:n], AF.Relu, bias=m1b[d]
                    )
                    off += n
                hidT.append(ht)

            # ---------------- phase 0: fc0 + gru0 (h0 = 0) ----------------
            x0_ps = pwe.tile([P, NPC], f32, name="x0_ps", tag="pwe")
            mm512(lambda o, n: x0_ps[0:32, o:o + n], fc0_wT,
                  lambda o, n: xT[:, o:o + n], NPC, True, True)
            x0r = wp.tile([32, NPC], f16, name="x0r")
            nc.scalar.activation(x0r[:], x0_ps[0:32, :], AF.Relu, bias=fc0_b)

            g0rz_ps = pwe.tile([P, NPC], f32, name="g0rz_ps", tag="pwe")
            mm512(lambda o, n: g0rz_ps[0:128, o:o + n], g0_wihT[:, 0:128],
                  lambda o, n: x0r[:, o:o + n], NPC, True, True)
            rz0 = wp.tile([P, NPC], f16, name="rz0", tag="gru_rz")
            nc.scalar.activation(rz0[:], g0rz_ps[0:128, :], AF.Sigmoid, bias=g0_brz)

            g0n_ps = pwe.tile([P, NPC], f32, name="g0n_ps", tag="pwe")
            mm512(lambda o, n: g0n_ps[0:64, o:o + n], g0_wihT[:, 128:192],
                  lambda o, n: x0r[:, o:o + n], NPC, True, True)
            gin0 = wp.tile([64, NPC], f16, name="gin0", tag="gru_gin")
            nc.scalar.activation(gin0[:], g0n_ps[0:64, :], AF.Identity, bias=g0_bihn)
            # h=0 so gh_n = bhh_n: broadcast bhh_n across columns (scale=0 trick)
            hn0 = wp.tile([64, NPC], f16, name="hn0", tag="gru_hn")
            nc.scalar.activation(hn0[:], gin0[:], AF.Identity, bias=g0_bhhn, scale=0.0)
            h_T = gru_elem(rz0, gin0, hn0, None, "p0")

            h_transposes(h_T, 0, "p0", want_sbuf=False)
            nc.gpsimd.collective_compute(
                "AllGather", OP.bypass, replica_groups=RG,
                ins=[hown[0].opt()], outs=[hfull[0].opt()],
            )

            # ---------------- conv depths ----------------
            h_nm = None
            for d in range(DEPTHS):
                aggT = pagg.tile([64, NPC], f32, name=f"aggT{d}", tag="agg")
                # root contribution first: start=True zeroes the accumulator
                for s in range(2):
                    nc.tensor.matmul(
                        aggT[0:64, s * 512:(s + 1) * 512],
                        rootw[d],
                        h_T[:, s * 512:(s + 1) * 512],
                        start=True, stop=False,
                    )

                # all gathers first (own-src ones lead: hown is ready
                # before the AllGather lands), so no GPSIMD multiply ever
                # blocks a queued gather or vice versa
                hsfs = {}
                for t in range(T):
                    hsf = hsfp.tile([P, DIM], f16, name=f"hsf{d}_{t}", tag="hsf")
                    src_dram = hown[d] if t < T_OWN else hfull[d]
                    nc.gpsimd.indirect_dma_start(
                        out=hsf[:], out_offset=None,
                        in_=src_dram[:, :],
                        in_offset=bass.IndirectOffsetOnAxis(ap=srcx[:, t:t + 1], axis=0),
                    )
                    hsfs[t] = hsf
                def emit_scatter(t):
                    for s in range(2):
                        nc.tensor.matmul(
                            aggT[0:64, s * 512:(s + 1) * 512],
                            msgs[t][:],
                            S[:, t * NPC + s * 512: t * NPC + (s + 1) * 512],
                            start=False, stop=(t == T - 1),
                        )

                msgs = {}
                for t in range(T):
                    hsf = hsfs[t]
                    hv = hsf[:, :].rearrange("p (g l) -> p g l", l=8)
                    wsb = wsbp.tile([P, 4096], f16, name=f"wsb{d}_{t}", tag="wsb")
                    for q in range(4):
                        wps = pwe.tile([P, NPC], f32, name=f"we{d}_{t}_{q}", tag="pwe")
                        mm512(lambda o, n, _q=q, _t=t: wps[:, o:o + n],
                              hidT[d][:, t * P:(t + 1) * P],
                              lambda o, n, _q=q: w2p[d][:, _q * 1024 + o:_q * 1024 + o + n],
                              1024, True, True)
                        nc.scalar.activation(
                            wsb[:, q * 1024:(q + 1) * 1024], wps[:], AF.Copy
                        )
                        if b2bc is not None:
                            nc.vector.tensor_tensor(
                                out=wsb[:, q * 1024:(q + 1) * 1024],
                                in0=wsb[:, q * 1024:(q + 1) * 1024],
                                in1=b2bc[d][:, q * 1024:(q + 1) * 1024], op=OP.add,
                            )
                    nc.vector.tensor_tensor(
                        out=wsb[:].rearrange("p (g o l) -> p g o l", o=64, l=8),
                        in0=wsb[:].rearrange("p (g o l) -> p g o l", o=64, l=8),
                        in1=hv[:, :, :].unsqueeze(2).to_broadcast([P, 8, 64, 8]),
                        op=OP.mult,
                    )
                    # fold-adds over the i_hi bits are contiguous-slice
                    # in-place adds (w2p column layout is (i_hi3, o, i_lo3))
                    for w in (2048, 1024, 512):
                        nc.vector.tensor_tensor(
                            out=wsb[:, 0:w], in0=wsb[:, 0:w], in1=wsb[:, w:2 * w],
                            op=OP.add,
                        )
                    msg = ep.tile([P, DIM], f16, name=f"msg{d}_{t}", tag="msg")
                    nc.vector.tensor_reduce(
                        out=msg[:], in_=wsb[:, 0:512].rearrange("p (o l) -> p o l", l=8),
                        axis=mybir.AxisListType.X, op=OP.add,
                    )
                    msgs[t] = msg
                    # scatter matmuls trail by SCAT_LAG tiles so one late msg
                    # never head-of-line-blocks the PE queue's We matmuls
                    if t >= SCAT_LAG:
                        emit_scatter(t - SCAT_LAG)
                for t in range(max(0, T - SCAT_LAG), T):
                    emit_scatter(t)
                xc = wp.tile([64, NPC], f16, name=f"xc{d}", tag="xc")
                nc.scalar.activation(xc[:], aggT[0:64, :], AF.Relu, bias=convb[d])

                # ---- GRU(xc, h), split into two column halves so the
                # serial DVE/ACT chain pipelines across halves
                rz = wp.tile([P, NPC], f16, name=f"rzs{d}", tag="gru_rz")
                z_s = wp.tile([64, NPC], f16, name=f"z_d{d}", tag="gru_z")
                gin = wp.tile([64, NPC], f16, name=f"gins{d}", tag="gru_gin")
                hn = wp.tile([64, NPC], f16, name=f"hns{d}", tag="gru_hn")
                t1 = wp.tile([64, NPC], f16, name=f"t1_d{d}", tag="gru_t1")
                nt = wp.tile([64, NPC], f16, name=f"nt_d{d}", tag="gru_nt")
                hm = wp.tile([64, NPC], f16, name=f"hm_d{d}", tag="gru_hm")
                hnew = wp.tile([64, NPC], f16, name=f"h_d{d}", tag="hT")
                for s in range(2):
                    sl = slice(s * 512, (s + 1) * 512)
                    ps1 = pwe.tile([P, 512], f32, name=f"rz{d}_{s}", tag="pwe")
                    nc.tensor.matmul(ps1[0:128, :], wihT[d][:, 0:128],
                                     xc[:, sl], start=True, stop=False)
                    nc.tensor.matmul(ps1[0:128, :], whhT[d][:, 0:128],
                                     h_T[:, sl], start=False, stop=True)
                    nc.scalar.activation(rz[:, sl], ps1[0:128, :], AF.Sigmoid, bias=brz[d])
                    nc.sync.dma_start(z_s[:, sl], rz[64:128, sl])
                    ps2 = pwe.tile([P, 512], f32, name=f"gin{d}_{s}", tag="pwe")
                    nc.tensor.matmul(ps2[0:64, :], wihT[d][:, 128:192],
                                     xc[:, sl], start=True, stop=True)
                    nc.scalar.activation(gin[:, sl], ps2[0:64, :], AF.Identity, bias=bihn[d])
                    ps3 = pwe.tile([P, 512], f32, name=f"ghn{d}_{s}", tag="pwe")
                    nc.tensor.matmul(ps3[0:64, :], whhT[d][:, 128:192],
                                     h_T[:, sl], start=True, stop=True)
                    nc.scalar.activation(hn[:, sl], ps3[0:64, :], AF.Identity, bias=bhhn[d])
                    nc.vector.tensor_tensor(out=t1[:, sl], in0=rz[0:64, sl],
                                            in1=hn[:, sl], op=OP.mult)
                    nc.vector.tensor_tensor(out=t1[:, sl], in0=t1[:, sl],
                                            in1=gin[:, sl], op=OP.add)
                    nc.scalar.activation(nt[:, sl], t1[:, sl], AF.Tanh)
                    nc.vector.tensor_tensor(out=hm[:, sl], in0=h_T[:, sl],
                                            in1=nt[:, sl], op=OP.subtract)
                    nc.vector.tensor_tensor(out=hm[:, sl], in0=hm[:, sl],
                                            in1=z_s[:, sl], op=OP.mult)
                    nc.vector.tensor_tensor(out=hnew[:, sl], in0=hm[:, sl],
                                            in1=nt[:, sl], op=OP.add)
                h_T = hnew

                if d < DEPTHS - 1:
                    h_transposes(h_T, d + 1, f"d{d}", want_sbuf=False)
                    nc.gpsimd.collective_compute(
                        "AllGather", OP.bypass, replica_groups=RG,
                        ins=[hown[d + 1].opt()], outs=[hfull[d + 1].opt()],
                    )
                else:
                    h_nm = h_transposes(h_T, None, f"d{d}", want_sbuf=True)
                    pooled_ps = pagg.tile([64, N_GRAPHS], f32, name="pooled_ps", tag="agg")
                    for c in range(8):
                        nc.tensor.matmul(
                            pooled_ps[0:64, :],
                            h_nm[c][:],
                            pS[:, c * N_GRAPHS:(c + 1) * N_GRAPHS],
                            start=(c == 0), stop=(c == 7),
                        )
                    pooled_sb = wp.tile([64, N_GRAPHS], f32, name="pooled_sb")
                    nc.scalar.activation(pooled_sb[:], pooled_ps[0:64, :], AF.Copy)
                    nc.sync.dma_start(ar_in[:, :], pooled_sb[:])

            # ---------------- pooling AllReduce + output MLP ----------------
            nc.gpsimd.collective_compute(
                "AllReduce", OP.add, replica_groups=RG,
                ins=[ar_in.opt()], outs=[ar_out.opt()],
            )
            pooled = wp.tile([64, N_GRAPHS], f32, name="pooled")
            nc.sync.dma_start(pooled[:], ar_out[:, :])
            pooled16 = wp.tile([64, N_GRAPHS], f16, name="pooled16")
            nc.scalar.activation(pooled16[:], pooled[:], AF.Copy)

            m1_ps = pagg.tile([64, N_GRAPHS], f32, name="m1_ps", tag="agg")
            nc.tensor.matmul(m1_ps[0:64, :], o0wT, pooled16[:], start=True, stop=True)
            m1r = wp.tile([64, N_GRAPHS], f16, name="m1r")
            nc.scalar.activation(m1r[:], m1_ps[0:64, :], AF.Relu, bias=o0b)

            m2_ps = pagg.tile([64, N_GRAPHS], f32, name="m2_ps", tag="agg")
            nc.tensor.matmul(m2_ps[0:32, :], o1wT, m1r[:], start=True, stop=True)
            m2b = wp.tile([32, N_GRAPHS], f16, name="m2b")
            nc.scalar.activation(m2b[:], m2_ps[0:32, :], AF.Identity, bias=o1b)

            m3_ps = pagg.tile([64, N_GRAPHS], f32, name="m3_ps", tag="agg")
            nc.tensor.matmul(m3_ps[0:1, :], o2wT, m2b[:], start=True, stop=True)
            ysb = wp.tile([1, N_GRAPHS], f32, name="ysb")
            nc.scalar.activation(ysb[:], m3_ps[0:1, :], AF.Identity, bias=o2b)
            nc.sync.dma_start(y_d[:, :], ysb[:])

    nc.finalize()
    return nc


def _prep(inputs):
    """Host-side sharding + weight permutation. Returns (T, T_OWN, b2_zero, in_maps)."""
    g = lambda k: np.asarray(inputs[k])
    x = g("x").astype(np.float32)
    ea = g("edge_attr").astype(np.float32)
    ei = g("edge_index").astype(np.int64)
    batch = g("batch").astype(np.int64)
    src, dst = ei[0], ei[1]

    owner = dst // NPC
    core_ids = [np.nonzero(owner == c)[0] for c in range(NC)]

    # own-src edges (src owned by the same core) are ordered first; they can
    # gather h from the core-local copy before the AllGather completes.
    own_lists, gen_lists = [], []
    for c in range(NC):
        ids = core_ids[c]
        is_own = (src[ids] // NPC) == c
        own_lists.append(ids[is_own])
        gen_lists.append(ids[~is_own])
    min_own = min(len(o) for o in own_lists)
    T_OWN = max(1, min(2, (min_own + P - 1) // P))
    n_own_slots = T_OWN * P

    seqs = []
    for c in range(NC):
        own, gen = own_lists[c], gen_lists[c]
        own_used = own[:n_own_slots]
        spill = own[n_own_slots:]
        gen_all = np.concatenate([spill, gen])
        seqs.append((own_used, gen_all))
    T_GEN = max((len(gl) + P - 1) // P for _, gl in seqs)
    T = T_OWN + T_GEN
    EP = T * P

    cnt = np.bincount(batch, minlength=N_GRAPHS).astype(np.float32)
    inv = 1.0 / np.maximum(cnt, 1.0)

    mlp2_b = g("mlp2_b").astype(np.float32)
    b2_zero = bool(np.all(mlp2_b == 0))

    # ---- shared weights: small tensors packed into two DMA payloads
    mlp1_w = g("mlp1_w").astype(np.float32)
    mlp1_b = g("mlp1_b").astype(np.float32)
    mlp2_w = g("mlp2_w").astype(np.float32)
    root_w = g("root_w").astype(np.float32)
    conv_b = g("conv_b").astype(np.float32)
    gru_wih = g("gru_wih").astype(np.float32)
    gru_whh = g("gru_whh").astype(np.float32)
    gru_bih = g("gru_bih").astype(np.float32)
    gru_bhh = g("gru_bhh").astype(np.float32)

    bias_pack = np.zeros((P, 22), np.float32)
    bj = [0]
    def bput(v):
        bias_pack[:len(v), bj[0]] = v; bj[0] += 1
    bput(g("fc0_b").astype(np.float32))
    bput((g("gru0_bih") + g("gru0_bhh")).astype(np.float32)[:128])
    bput(g("gru0_bih").astype(np.float32)[128:])
    bput(g("gru0_bhh").astype(np.float32)[128:])
    for d in range(DEPTHS): bput(mlp1_b[d])
    for d in range(DEPTHS): bput(conv_b[d])
    for d in range(DEPTHS): bput((gru_bih[d] + gru_bhh[d])[:128])
    for d in range(DEPTHS): bput(gru_bih[d][128:])
    for d in range(DEPTHS): bput(gru_bhh[d][128:])
    bput(g("out0_b").astype(np.float32))
    bput(g("out1_b").astype(np.float32))
    bput(g("out2_b").astype(np.float32))

    wpack = np.zeros((64, 2049), np.float16)
    wj = [0]
    def wput(m):
        r, c = m.shape
        wpack[:r, wj[0]:wj[0] + c] = m.astype(np.float16); wj[0] += c
    wput(g("fc0_w").astype(np.float32).T)
    wput(g("gru0_wih").astype(np.float32).T)
    for d in range(DEPTHS): wput(mlp1_w[d].T)
    for d in range(DEPTHS): wput(root_w[d])
    for d in range(DEPTHS): wput(gru_wih[d].T)
    for d in range(DEPTHS): wput(gru_whh[d].T)
    wput(g("out0_w").astype(np.float32).T)
    wput(g("out1_w").astype(np.float32).T)
    wput(g("out2_w").astype(np.float32).T)

    shared = {"biaspack": bias_pack, "wpack16": wpack}
    for d in range(DEPTHS):
        # column layout (i_hi3, o, i_lo3): fold-adds over i become
        # contiguous-slice adds (DVE 2x mode needs packed operands)
        shared[f"w2p{d}"] = (
            mlp2_w[d].reshape(8, 8, 64, 128).transpose(3, 0, 2, 1).reshape(128, 4096)
        ).astype(np.float16)
        if not b2_zero:
            b2p = mlp2_b[d].reshape(8, 8, 64).transpose(0, 2, 1).reshape(4096)
            shared[f"b2bc{d}"] = np.broadcast_to(
                b2p.astype(np.float16), (P, 4096)
            ).copy()

    in_maps = []
    for c in range(NC):
        own_used, gen_all = seqs[c]
        ids = np.concatenate([own_used, np.full(n_own_slots - len(own_used), -1),
                              gen_all, np.full(EP - n_own_slots - len(gen_all), -1)])
        valid = ids >= 0
        idv = ids.copy()
        idv[~valid] = 0  # placeholder edge (zero S column kills contribution)
        src_pad = src[idv].astype(np.int32)
        src_pad[~valid] = c * NPC  # any in-range node
        # own-src tiles use LOCAL indices into hown
        src_pad[:n_own_slots] -= c * NPC
        ea_pad = ea[idv].astype(np.float32)
        ea_pad[~valid] = 0.0
        S_full = np.zeros((EP, NPC), np.float16)
        rows = np.nonzero(valid)[0]
        S_full[rows, dst[idv[rows]] - c * NPC] = 1.0
        S_tab = np.zeros((P, T * NPC), np.float16)
        for t in range(T):
            S_tab[:, t * NPC:(t + 1) * NPC] = S_full[t * P:(t + 1) * P]
        pm_full = np.zeros((NPC, N_GRAPHS), np.float16)
        nb = batch[c * NPC:(c + 1) * NPC]
        pm_full[np.arange(NPC), nb] = inv[nb].astype(np.float16)
        # pre-chunked [(128), 8*64]: chunk cc covers nodes cc*128..+128
        pm = np.zeros((P, 8 * N_GRAPHS), np.float16)
        for cc in range(8):
            pm[:, cc * N_GRAPHS:(cc + 1) * N_GRAPHS] = pm_full[cc * P:(cc + 1) * P]
        m = {
            "xT": x[c * NPC:(c + 1) * NPC].T.astype(np.float16).copy(),
            "eaT": ea_pad.T.astype(np.float16).copy(),
            "srcidx": src_pad.reshape(T, P).T.copy(),
            "S": S_tab,
            "poolS": pm,
        }
        m.update(shared)
        in_maps.append(m)
    return T, T_OWN, b2_zero, in_maps


def kernel(**inputs) -> np.ndarray:
    global LAST_EXEC_NS, LAST_RESULTS
    T, T_OWN, b2_zero, in_maps = _prep(inputs)
    key = (T, T_OWN, b2_zero)
    if key not in _CACHE:
        _CACHE[key] = _build(T, T_OWN, b2_zero)
    nc = _CACHE[key]

    from concourse.bass_utils import run_bass_kernel_spmd

    if TRACE:
        res = run_bass_kernel_spmd(
            nc, in_maps, list(range(NC)), trace=True, trace_cores=list(range(NC))
        )
        LAST_EXEC_NS = res.exec_time_ns
        LAST_RESULTS = res
    else:
        res = run_bass_kernel_spmd(nc, in_maps, list(range(NC)))
    return res.results[0]["y"].reshape(N_GRAPHS).astype(np.float32)
